# revision 10
# baseline (speedup 1.0000x reference)
"""Multi-head causal attention (B=2, T=2048, D=2048, H=16) on 8 trn2 NeuronCores.

Sharding: tensor-parallel over heads (2 heads/core). x^T is replicated, W_qkv
column-sliced and W_out row-sliced per core; each core computes a full-shape
partial of the output projection and the host sums the 8 partials (+ b_out).

v2: bf16 end-to-end. x/W_qkv/W_out and all intermediates (q,k,v,P,O) are bf16
(fp32 PSUM accumulation), halving HBM traffic and SBUF footprint; rel err
~6e-3 vs the 2e-2 gate. Scores are computed transposed [k, q] so softmax
rowsums need a ones-matmul; exp runs without max-subtraction (scores < ~25).
Causal structure: only lower-triangular tiles, with diagonal blocks shrunk to
N in {512,384,256,128} and the residual triangle zeroed by a DVE multiply
with a constant mask. The output projection is emitted per 512-token block
right after its O^T tiles are normalized, spreading stage-3 matmuls and y
DMAs across the attention phase instead of a serial tail. Engine placement:
ACT = exp + q/v bias evictions + half the y evictions; DVE = RoPE, k bias,
diag mask, normalization; Pool(gpsimd) = V^T->V eviction copies + half the y
evictions; all DMA on SP.
"""

import math
import os

import numpy as np

import concourse.bass as bass
import concourse.mybir as mybir
import concourse.tile as tile
from concourse import bacc
from concourse.bass_utils import run_bass_kernel_spmd

B, T, D_IN, D_MODEL, H = 2, 2048, 2048, 2048, 16
DH = 128
NCORES = 8
HPC = H // NCORES  # heads per core
BT = B * T
SCALE = 1.0 / math.sqrt(DH)

F32 = mybir.dt.float32
BF16 = mybir.dt.bfloat16
AF = mybir.ActivationFunctionType
ALU = mybir.AluOpType

TOKT = 512             # stage-1 token tile
NTT = T // TOKT        # token tiles per batch (4)
NDCH = D_IN // 128     # d_in contraction chunks (16)
NQ = T // 128          # 128-token chunks per batch (16)
NJ = T // 512          # q 512-tiles per batch (4)
NFT = D_MODEL // 512   # output feature tiles (4)


def build_nc(debug=False, reps=1):
    nc = bacc.Bacc("TRN2", target_bir_lowering=False, debug=False,
                   num_devices=NCORES)

    xT = nc.dram_tensor("xT", [D_IN, BT], BF16, kind="ExternalInput")
    wq = nc.dram_tensor("wq", [D_IN, HPC * DH], BF16, kind="ExternalInput")
    wk = nc.dram_tensor("wk", [D_IN, HPC * DH], BF16, kind="ExternalInput")
    wv = nc.dram_tensor("wv", [D_IN, HPC * DH], BF16, kind="ExternalInput")
    bq = nc.dram_tensor("bq", [HPC * DH], F32, kind="ExternalInput")
    bk = nc.dram_tensor("bk", [HPC * DH], F32, kind="ExternalInput")
    bv = nc.dram_tensor("bv", [HPC * DH], F32, kind="ExternalInput")
    wo = nc.dram_tensor("wo", [HPC * DH, D_MODEL], BF16, kind="ExternalInput")
    cosT = nc.dram_tensor("cosT", [DH, T], BF16, kind="ExternalInput")
    sinSW = nc.dram_tensor("sinSW", [DH, T], BF16, kind="ExternalInput")
    ident_d = nc.dram_tensor("ident", [128, 128], BF16, kind="ExternalInput")
    tri_d = nc.dram_tensor("tri", [128, 128], BF16, kind="ExternalInput")
    ones1_d = nc.dram_tensor("ones1", [1, 128], BF16, kind="ExternalInput")
    onescol_d = nc.dram_tensor("onescol", [128, 1], BF16, kind="ExternalInput")
    y = nc.dram_tensor("y", [BT, D_MODEL], BF16, kind="ExternalOutput")

    dbg = {}
    if debug:
        dbg["qT"] = nc.dram_tensor("dbg_qT", [HPC, B, DH, T], F32, kind="ExternalOutput")
        dbg["kT"] = nc.dram_tensor("dbg_kT", [HPC, B, DH, T], F32, kind="ExternalOutput")
        dbg["v"] = nc.dram_tensor("dbg_v", [B, T, HPC * DH], F32, kind="ExternalOutput")
        dbg["ot"] = nc.dram_tensor("dbg_ot", [B, HPC, DH, T], F32, kind="ExternalOutput")

    with tile.TileContext(nc) as tc:
        with (
            tc.tile_pool(name="persist", bufs=1) as pp,
            tc.tile_pool(name="weights", bufs=1) as wp,
            tc.tile_pool(name="qkv", bufs=1) as qp,
        ):
            # ---- per-core weights (persistent, outside the rep loop)
            wq_sb = wp.tile([128, NDCH, HPC * DH], BF16, name="wq_sb")
            wk_sb = wp.tile([128, NDCH, HPC * DH], BF16, name="wk_sb")
            wv_sb = wp.tile([128, NDCH, HPC * DH], BF16, name="wv_sb")
            wo_sb = wp.tile([128, HPC, D_MODEL], BF16, name="wo_sb")

            for hf in range(4):
                for t_, d_ in ((wq_sb, wq), (wk_sb, wk), (wv_sb, wv)):
                    nc.sync.dma_start(
                        t_[:, hf * (NDCH // 4):(hf + 1) * (NDCH // 4), :],
                        d_.ap()[hf * (D_IN // 4):(hf + 1) * (D_IN // 4), :]
                        .rearrange("(c p) f -> p c f", p=128))
            nc.sync.dma_start(wo_sb[:],
                              wo.ap().rearrange("(h p) f -> p h f", p=128))

            cosT_sb = pp.tile([DH, T], BF16, name="cosT_sb")
            sinSW_sb = pp.tile([DH, T], BF16, name="sinSW_sb")
            nc.sync.dma_start(cosT_sb[:], cosT.ap())
            nc.sync.dma_start(sinSW_sb[:], sinSW.ap())
            ones1 = pp.tile([1, 128], BF16, name="ones1")
            onescol = pp.tile([128, 1], BF16, name="onescol")
            ident = pp.tile([128, 128], BF16, name="ident")
            tri = pp.tile([128, 128], BF16, name="tri")
            nc.sync.dma_start(ones1[:], ones1_d.ap())
            nc.sync.dma_start(onescol[:], onescol_d.ap())
            nc.sync.dma_start(ident[:], ident_d.ap())
            nc.sync.dma_start(tri[:], tri_d.ap())
            bqt = pp.tile([128, HPC], F32, name="bqt")
            bkt = pp.tile([128, HPC], F32, name="bkt")
            bvt = pp.tile([128, HPC], F32, name="bvt")
            nc.sync.dma_start(bqt[:], bq.ap().rearrange("(h d) -> d h", d=DH))
            nc.sync.dma_start(bkt[:], bk.ap().rearrange("(h d) -> d h", d=DH))
            nc.sync.dma_start(bvt[:], bv.ap().rearrange("(h d) -> d h", d=DH))

            # ---- per-batch Q^T/K^T/V and O^T buffers (persistent)
            qT_sb = [qp.tile([DH, T], BF16, name=f"qT{h}") for h in range(HPC)]
            kT_sb = [qp.tile([DH, T], BF16, name=f"kT{h}") for h in range(HPC)]
            v_sb = qp.tile([128, NQ, HPC * DH], BF16, name="v_sb")
            ot_sb = [[qp.tile([DH, T], BF16, name=f"ot{b}_{h}") for h in range(HPC)]
                     for b in range(B)]

            import contextlib
            rep_ctx = (tc.For_i(0, reps, 1, hint_engines=(
                mybir.EngineType.PE, mybir.EngineType.Activation,
                mybir.EngineType.DVE, mybir.EngineType.Pool,
                mybir.EngineType.SP))
                if reps > 1 else contextlib.nullcontext())
            with rep_ctx:
                _emit_body(nc, tc, xT, wq_sb, wk_sb, wv_sb, bqt, bkt, bvt,
                           cosT_sb, sinSW_sb, qT_sb, kT_sb, v_sb, ot_sb,
                           wo_sb, y, ones1, onescol, ident, tri, dbg)
    nc.compile()
    return nc


def _emit_body(nc, tc, xT, wq_sb, wk_sb, wv_sb, bqt, bkt, bvt, cosT_sb,
               sinSW_sb, qT_sb, kT_sb, v_sb, ot_sb, wo_sb, y, ones1,
               onescol, ident, tri, dbg):
    with (
        tc.tile_pool(name="xs", bufs=3) as xs,
        tc.tile_pool(name="y_st", bufs=3) as ysp,
    ):
        for b in range(B):
            _stage1(nc, tc, b, xT, wq_sb, wk_sb, wv_sb, bqt, bkt, bvt,
                    cosT_sb, sinSW_sb, qT_sb, kT_sb, v_sb, ident, xs)
            if dbg:
                for h in range(HPC):
                    nc.sync.dma_start(dbg["qT"].ap()[h, b],
                                      qT_sb[h][:])
                    nc.sync.dma_start(dbg["kT"].ap()[h, b],
                                      kT_sb[h][:])
                nc.sync.dma_start(
                    dbg["v"].ap()[b].rearrange("(c p) f -> p c f", p=128),
                    v_sb[:])
            _stage23(nc, tc, b, qT_sb, kT_sb, v_sb, ones1, onescol, tri,
                     ot_sb, wo_sb, y, ysp, dbg)
        if dbg:
            for bb in range(B):
                for h in range(HPC):
                    nc.sync.dma_start(dbg["ot"].ap()[bb, h], ot_sb[bb][h][:])


def _stage1(nc, tc, b, xT, wq_sb, wk_sb, wv_sb, bqt, bkt, bvt,
            cosT_sb, sinSW_sb, qT_sb, kT_sb, v_sb, ident, xs):
    """QKV projection + RoPE for batch b: fills qT_sb/kT_sb/v_sb (bf16).

    Loop nest is d_in-chunk-outer so each x^T quarter-tile is touched once.
    q/k/v are computed transposed ([feat, tok], N=512); V is rotated back to
    natural [tok, feat] layout with PE transposes (stationary operand of P@V).
    RoPE: t1 = stg*cos, t2 = stg*sinSW (sign/swap folded into the table),
    out halves = t1_half + t2_otherhalf  -- all on DVE, no staging copies.
    """
    with (
        tc.tile_pool(name="st", bufs=3) as st,
        tc.tile_pool(name="vt", bufs=2) as vtp,
        tc.tile_pool(name="ps_qk", bufs=4, space="PSUM") as psqk,
        tc.tile_pool(name="ps_v", bufs=2, space="PSUM") as psv,
        tc.tile_pool(name="ps_tr", bufs=2, space="PSUM") as pstr,
    ):
        for tau in range(NTT):
            pos = tau * TOKT
            gtok = b * T + pos
            accs = [psqk.tile([128, TOKT], F32, name="qk_acc") for _ in range(4)]
            accvT = [psv.tile([128, TOKT], F32, name="vT_acc") for _ in range(2)]
            for quarter in range(4):
                xt = xs.tile([128, 4, TOKT], BF16, name="xt")
                nc.sync.dma_start(
                    xt[:],
                    xT.ap()[quarter * 512:(quarter + 1) * 512,
                            gtok:gtok + TOKT]
                    .rearrange("(c p) t -> p c t", p=128))
                for cl in range(4):
                    c = quarter * 4 + cl
                    for fi, (wsb, hh) in enumerate(
                            ((wq_sb, 0), (wq_sb, 1), (wk_sb, 0), (wk_sb, 1))):
                        nc.tensor.matmul(
                            accs[fi][:], wsb[:, c, hh * DH:(hh + 1) * DH],
                            xt[:, cl, :],
                            start=(c == 0), stop=(c == NDCH - 1))
                    for hh in range(HPC):
                        nc.tensor.matmul(
                            accvT[hh][:], wv_sb[:, c, hh * DH:(hh + 1) * DH],
                            xt[:, cl, :],
                            start=(c == 0), stop=(c == NDCH - 1))
            # q/k evictions with bias (split ACT/DVE), then RoPE on DVE
            for fi, (bias, dest, hh) in enumerate(
                    ((bqt, qT_sb, 0), (bqt, qT_sb, 1),
                     (bkt, kT_sb, 0), (bkt, kT_sb, 1))):
                stg = st.tile([128, TOKT], BF16, name="stg")
                if fi < 2:
                    nc.scalar.activation(stg[:], accs[fi][:], AF.Identity,
                                         bias=bias[:, hh:hh + 1], scale=1.0)
                else:
                    nc.vector.tensor_scalar_add(stg[:], accs[fi][:],
                                                bias[:, hh:hh + 1])
                rot = st.tile([128, TOKT], BF16, name="rot")
                nc.scalar.copy(rot[0:64, :], stg[64:128, :])
                nc.vector.tensor_copy(rot[64:128, :], stg[0:64, :])
                nc.vector.tensor_tensor(
                    stg[:], stg[:], cosT_sb[:, pos:pos + TOKT], ALU.mult)
                nc.vector.tensor_tensor(
                    rot[:], rot[:], sinSW_sb[:, pos:pos + TOKT], ALU.mult)
                nc.vector.tensor_tensor(
                    dest[hh][:, pos:pos + TOKT], stg[:], rot[:], ALU.add)
            # V: evict V^T with bias (ACT), PE-transpose to natural layout,
            # land in v_sb via Pool copies
            for hh in range(HPC):
                vt = vtp.tile([128, TOKT], BF16, name="vt")
                nc.scalar.activation(vt[:], accvT[hh][:], AF.Identity,
                                     bias=bvt[:, hh:hh + 1], scale=1.0)
                for ts in range(4):
                    tr = pstr.tile([128, 128], BF16, name="tr")
                    nc.tensor.transpose(tr[:], vt[:, ts * 128:(ts + 1) * 128],
                                        ident[:])
                    nc.vector.tensor_copy(
                        v_sb[:, (pos // 128) + ts, hh * DH:(hh + 1) * DH],
                        tr[:])


def _stage23(nc, tc, b, qT_sb, kT_sb, v_sb, ones1, onescol, tri, ot_sb,
             wo_sb, y, ysp, dbg):
    """Causal attention for batch b + per-j output projection emission.

    Per (j, kk, h): S^T -> exp (ACT, bf16 out) -> diag-triangle zero (DVE
    mask multiply) -> P@V + ones-rowsum accumulation on PE. Diagonal blocks
    shrink N to 512-128*(kk%4). After each j: reciprocal + PE-broadcast of
    1/rowsum, O^T normalized on DVE; then the output projection for these
    512 tokens runs immediately (interleaving its matmuls and y DMAs with
    the next j's attention).
    """
    with (
        tc.tile_pool(name="spsB", bufs=2, space="PSUM") as spsB,
        tc.tile_pool(name="rps", bufs=2, space="PSUM") as rps,
        tc.tile_pool(name="ops", bufs=2, space="PSUM") as ops,
        tc.tile_pool(name="y_ps", bufs=2, space="PSUM") as yps,
        tc.tile_pool(name="scr", bufs=4) as scr,
        tc.tile_pool(name="pt_p", bufs=5) as ptp,
    ):
        for j in range(NJ):
            nkk = 4 * j + 4
            rp = [rps.tile([1, 512], F32, name="r_ps") for _ in range(HPC)]
            op = [ops.tile([128, 512], F32, name="o_ps") for _ in range(HPC)]
            for kk in range(nkk):
                diag = (kk // 4 == j)
                off = (kk % 4) * 128 if diag else 0
                n = 512 - off
                qlo = j * 512 + off
                for h in range(HPC):
                    qT, kT = qT_sb[h], kT_sb[h]
                    sp = spsB.tile([128, 512], F32, name="st_ps", tag="st_ps")
                    nc.tensor.matmul(sp[:, 0:n],
                                     kT[:, kk * 128:(kk + 1) * 128],
                                     qT[:, qlo:(j + 1) * 512],
                                     start=True, stop=True)
                    pt = ptp.tile([128, 512], BF16, name="pt")
                    nc.scalar.activation(pt[:, 0:n], sp[:, 0:n], AF.Exp,
                                         bias=0.0, scale=SCALE)
                    if diag:
                        # zero q < k inside the leading 128-col block
                        nc.vector.tensor_tensor(pt[:, 0:128], pt[:, 0:128],
                                                tri[:], ALU.mult)
                    nc.tensor.matmul(op[h][:, off:512],
                                     v_sb[:, kk, h * DH:(h + 1) * DH],
                                     pt[:, 0:n], start=(kk == 0),
                                     stop=(kk == nkk - 1))
                    nc.tensor.matmul(rp[h][:, off:512], onescol[:],
                                     pt[:, 0:n], start=(kk == 0),
                                     stop=(kk == nkk - 1))
            # rowsum -> reciprocal -> broadcast across partitions -> evict
            for h in range(HPC):
                rrow_inv = scr.tile([1, 512], BF16, name="rrow_inv")
                with nc.allow_low_precision(reason="softmax denom to bf16"):
                    nc.vector.reciprocal(rrow_inv[:], rp[h][:])
                rb_ps = spsB.tile([128, 512], F32, name="st_ps", tag="st_ps")
                nc.tensor.matmul(rb_ps[:], ones1[:], rrow_inv[:],
                                 start=True, stop=True)
                rb = scr.tile([128, 512], F32, name="rb")
                nc.scalar.copy(rb[:], rb_ps[:])
                nc.vector.tensor_tensor(ot_sb[b][h][:, j * 512:(j + 1) * 512],
                                        op[h][:], rb[:], ALU.mult)
            # ---- output projection for tokens [j*512, (j+1)*512) ----------
            for tt in range(4):
                trow = j * 4 + tt
                yst = ysp.tile([128, D_MODEL], BF16, name="y_st")
                for ft in range(NFT):
                    ps = yps.tile([128, 512], F32, name="y_acc")
                    for h in range(HPC):
                        nc.tensor.matmul(
                            ps[:], ot_sb[b][h][:, trow * 128:(trow + 1) * 128],
                            wo_sb[:, h, ft * 512:(ft + 1) * 512],
                            start=(h == 0), stop=(h == HPC - 1))
                    if ft % 2 == 0:
                        nc.scalar.copy(yst[:, ft * 512:(ft + 1) * 512], ps[:])
                    else:
                        nc.vector.tensor_copy(yst[:, ft * 512:(ft + 1) * 512],
                                              ps[:])
                nc.sync.dma_start(
                    y.ap()[b * T + trow * 128:b * T + (trow + 1) * 128, :],
                    yst[:])


_CACHE = {}


def _get_nc():
    if "nc" not in _CACHE:
        _CACHE["nc"] = build_nc(debug=bool(int(os.environ.get("KERNEL_DEBUG", "0"))))
    return _CACHE["nc"]


def _host_prep(x, W_qkv, b_qkv, W_out, mask):
    from ml_dtypes import bfloat16
    xT = np.ascontiguousarray(x.reshape(BT, D_IN).T.astype(bfloat16))
    Wr = W_qkv.reshape(D_IN, H, 3, DH)
    br = b_qkv.reshape(H, 3, DH)
    # RoPE tables, transposed, sign-folded (rows 0:64 of sin negated) for the
    # half-swap rotate: q_rot = q*cos + swap_halves(q)*sinSW.
    inv_freq = (1.0 / (10000.0 ** (np.arange(0, DH, 2, dtype=np.float32) / DH))).astype(np.float32)
    tpos = np.arange(T, dtype=np.float32)
    freqs = tpos[:, None] * inv_freq[None, :]              # (T, 64)
    emb = np.concatenate([freqs, freqs], axis=-1)          # (T, 128)
    cosT = np.ascontiguousarray(np.cos(emb).astype(np.float32).T)
    sinSW = np.sin(emb).astype(np.float32).T               # (128, T)
    sinSW[0:64] = -sinSW[0:64]

    ident = np.eye(128, dtype=np.float32)
    tri_m = (np.arange(128)[None, :] >= np.arange(128)[:, None]).astype(np.float32)
    ones1 = np.ones((1, 128), dtype=np.float32)
    onescol = np.ones((128, 1), dtype=np.float32)

    in_maps = []
    for i in range(NCORES):
        hs = [HPC * i + k for k in range(HPC)]
        in_maps.append({
            "xT": xT,
            "wq": np.ascontiguousarray(Wr[:, hs, 0, :].reshape(D_IN, HPC * DH).astype(bfloat16)),
            "wk": np.ascontiguousarray(Wr[:, hs, 1, :].reshape(D_IN, HPC * DH).astype(bfloat16)),
            "wv": np.ascontiguousarray(Wr[:, hs, 2, :].reshape(D_IN, HPC * DH).astype(bfloat16)),
            "bq": np.ascontiguousarray(br[hs, 0, :].reshape(HPC * DH)),
            "bk": np.ascontiguousarray(br[hs, 1, :].reshape(HPC * DH)),
            "bv": np.ascontiguousarray(br[hs, 2, :].reshape(HPC * DH)),
            "wo": np.ascontiguousarray(W_out[hs[0] * DH:(hs[-1] + 1) * DH, :].astype(bfloat16)),
            "cosT": np.ascontiguousarray(cosT.astype(bfloat16)),
            "sinSW": np.ascontiguousarray(sinSW.astype(bfloat16)),
            "ident": ident.astype(bfloat16),
            "tri": tri_m.astype(bfloat16),
            "ones1": ones1.astype(bfloat16),
            "onescol": onescol.astype(bfloat16),
        })
    return in_maps


def kernel(x, W_qkv, b_qkv, W_out, b_out, mask):
    x = np.asarray(x, dtype=np.float32)
    in_maps = _host_prep(np.asarray(x), np.asarray(W_qkv), np.asarray(b_qkv),
                         np.asarray(W_out), np.asarray(mask))
    nc = _get_nc()
    res = run_bass_kernel_spmd(nc, in_maps, core_ids=list(range(NCORES)))
    out = np.asarray(res.results[0]["y"], dtype=np.float32)
    for i in range(1, NCORES):
        out += np.asarray(res.results[i]["y"], dtype=np.float32)
    out += np.asarray(b_out, dtype=np.float32)[None, :]
    return out.reshape(B, T, D_MODEL).astype(np.float32)


# revision 13
# speedup vs baseline: 1.3219x; 1.3219x over previous
"""Multi-head causal attention (B=2, T=2048, D=2048, H=16) on 8 trn2 NeuronCores.

Sharding: tensor-parallel over heads (2 heads/core). x^T is replicated, W_qkv
column-sliced and W_out row-sliced per core; each core computes a full-shape
partial of the output projection and the host sums the 8 partials (+ b_out).

v3: bf16 end-to-end (fp32 PSUM accumulation; rel err ~7e-3 vs the 2e-2 gate)
and, critically, every PE matmul sequence is arranged to stay on one PSUM
bank for >=2 consecutive matmuls: HW measurement shows accumulation streams
that switch banks every matmul run at ~790ns/MM vs ~215-240ns for runs>=2
(unmodeled by CoreSim).

Stage 1 (QKV+RoPE): per 512-token tile, the four transposed projections
(q0,q1,k0,k1) run as 8-matmul same-bank bursts per half-x-tile; V is computed
directly in natural [token, feature] layout by making the x chunk the
stationary operand (out = x_chunk.T @ W_v slice), which kills the PE
transposes and their eviction copies entirely. Stage 2: scores transposed
[k, q], raw exp (scores < ~25, no max subtraction), denominators via
ones-column matmuls; kk-steps processed in pairs with P@V / rowsum matmuls
software-pipelined one pair behind the S-matmul+exp of the next pair, so PV
and rowsum hit their accumulator banks in runs of 2 while exp latency hides.
Full S tiles are split into two N=256 matmuls into the same bank. Diagonal
blocks shrink N to 512-128*(kk%4); the residual 128-col triangle is zeroed by
a DVE multiply with a constant mask. The output projection for each 512-token
block is emitted right after its O^T normalization, reusing the S PSUM pool.
Engine placement: ACT = exp + q bias + rotate-half copy + rb + half the y
evictions; DVE = RoPE muls, k bias, V/ot evictions, diag mask, reciprocal,
the other half of y; all DMA on SP; GpSimd unused (slow on HW, cannot read
PSUM).
"""

import math
import os

import numpy as np

import concourse.bass as bass
import concourse.mybir as mybir
import concourse.tile as tile
from concourse import bacc
from concourse.bass_utils import run_bass_kernel_spmd

B, T, D_IN, D_MODEL, H = 2, 2048, 2048, 2048, 16
DH = 128
NCORES = 8
HPC = H // NCORES  # heads per core
BT = B * T
SCALE = 1.0 / math.sqrt(DH)

F32 = mybir.dt.float32
BF16 = mybir.dt.bfloat16
AF = mybir.ActivationFunctionType
ALU = mybir.AluOpType

TOKT = 512             # stage-1 token tile
NTT = T // TOKT        # token tiles per batch (4)
NDCH = D_IN // 128     # d_in contraction chunks (16)
NQ = T // 128          # 128-token chunks per batch (16)
NJ = T // 512          # q 512-tiles per batch (4)
NFT = D_MODEL // 512   # output feature tiles (4)


def build_nc(debug=False, reps=1, stages=None):
    stages = stages or os.environ.get("KSTAGES", "all")
    nc = bacc.Bacc("TRN2", target_bir_lowering=False, debug=False,
                   num_devices=NCORES)

    xT = nc.dram_tensor("xT", [D_IN, BT], BF16, kind="ExternalInput")
    wq = nc.dram_tensor("wq", [D_IN, HPC * DH], BF16, kind="ExternalInput")
    wk = nc.dram_tensor("wk", [D_IN, HPC * DH], BF16, kind="ExternalInput")
    wv = nc.dram_tensor("wv", [D_IN, HPC * DH], BF16, kind="ExternalInput")
    bq = nc.dram_tensor("bq", [HPC * DH], F32, kind="ExternalInput")
    bk = nc.dram_tensor("bk", [HPC * DH], F32, kind="ExternalInput")
    bvf = nc.dram_tensor("bvf", [128, HPC * DH], BF16, kind="ExternalInput")
    wo = nc.dram_tensor("wo", [HPC * DH, D_MODEL], BF16, kind="ExternalInput")
    cosT = nc.dram_tensor("cosT", [DH, T], BF16, kind="ExternalInput")
    sinSW = nc.dram_tensor("sinSW", [DH, T], BF16, kind="ExternalInput")
    tri_d = nc.dram_tensor("tri", [128, 128], BF16, kind="ExternalInput")
    ones1_d = nc.dram_tensor("ones1", [1, 128], BF16, kind="ExternalInput")
    onescol_d = nc.dram_tensor("onescol", [128, 1], BF16, kind="ExternalInput")
    y = nc.dram_tensor("y", [BT, D_MODEL], BF16, kind="ExternalOutput")

    dbg = {}
    if debug:
        dbg["qT"] = nc.dram_tensor("dbg_qT", [HPC, B, DH, T], F32, kind="ExternalOutput")
        dbg["kT"] = nc.dram_tensor("dbg_kT", [HPC, B, DH, T], F32, kind="ExternalOutput")
        dbg["v"] = nc.dram_tensor("dbg_v", [B, T, HPC * DH], F32, kind="ExternalOutput")
        dbg["ot"] = nc.dram_tensor("dbg_ot", [B, HPC, DH, T], F32, kind="ExternalOutput")

    with tile.TileContext(nc) as tc:
        with (
            tc.tile_pool(name="persist", bufs=1) as pp,
            tc.tile_pool(name="weights", bufs=1) as wp,
            tc.tile_pool(name="qkv", bufs=1) as qp,
        ):
            # ---- per-core weights (persistent, outside the rep loop)
            wq_sb = wp.tile([128, NDCH, HPC * DH], BF16, name="wq_sb")
            wk_sb = wp.tile([128, NDCH, HPC * DH], BF16, name="wk_sb")
            wv_sb = wp.tile([128, NDCH, HPC * DH], BF16, name="wv_sb")
            wo_sb = wp.tile([128, HPC, D_MODEL], BF16, name="wo_sb")

            for hf in range(4):
                for t_, d_ in ((wq_sb, wq), (wk_sb, wk), (wv_sb, wv)):
                    nc.sync.dma_start(
                        t_[:, hf * (NDCH // 4):(hf + 1) * (NDCH // 4), :],
                        d_.ap()[hf * (D_IN // 4):(hf + 1) * (D_IN // 4), :]
                        .rearrange("(c p) f -> p c f", p=128))
            nc.sync.dma_start(wo_sb[:],
                              wo.ap().rearrange("(h p) f -> p h f", p=128))

            cosT_sb = pp.tile([DH, T], BF16, name="cosT_sb")
            sinSW_sb = pp.tile([DH, T], BF16, name="sinSW_sb")
            nc.sync.dma_start(cosT_sb[:], cosT.ap())
            nc.sync.dma_start(sinSW_sb[:], sinSW.ap())
            ones1 = pp.tile([1, 128], BF16, name="ones1")
            onescol = pp.tile([128, 1], BF16, name="onescol")
            tri = pp.tile([128, 128], BF16, name="tri")
            nc.sync.dma_start(ones1[:], ones1_d.ap())
            nc.sync.dma_start(onescol[:], onescol_d.ap())
            nc.sync.dma_start(tri[:], tri_d.ap())
            bqt = pp.tile([128, HPC], F32, name="bqt")
            bkt = pp.tile([128, HPC], F32, name="bkt")
            bv_full = pp.tile([128, HPC * DH], BF16, name="bv_full")
            nc.sync.dma_start(bqt[:], bq.ap().rearrange("(h d) -> d h", d=DH))
            nc.sync.dma_start(bkt[:], bk.ap().rearrange("(h d) -> d h", d=DH))
            nc.sync.dma_start(bv_full[:], bvf.ap())

            # ---- per-batch Q^T/K^T/V and O^T buffers (persistent)
            qT_sb = [qp.tile([DH, T], BF16, name=f"qT{h}") for h in range(HPC)]
            kT_sb = [qp.tile([DH, T], BF16, name=f"kT{h}") for h in range(HPC)]
            v_sb = qp.tile([128, NQ, HPC * DH], BF16, name="v_sb")
            ot_sb = [[qp.tile([DH, T], BF16, name=f"ot{b}_{h}") for h in range(HPC)]
                     for b in range(B)]
            if stages == "s23":
                for h in range(HPC):
                    nc.gpsimd.memset(qT_sb[h][:], 0.5)
                    nc.gpsimd.memset(kT_sb[h][:], 0.5)
                nc.gpsimd.memset(v_sb[:], 0.5)

            import contextlib
            rep_ctx = (tc.For_i(0, reps, 1, hint_engines=(
                mybir.EngineType.PE, mybir.EngineType.Activation,
                mybir.EngineType.DVE, mybir.EngineType.Pool,
                mybir.EngineType.SP))
                if reps > 1 else contextlib.nullcontext())
            with rep_ctx:
                _emit_body(nc, tc, xT, wq_sb, wk_sb, wv_sb, bqt, bkt, bv_full,
                           cosT_sb, sinSW_sb, qT_sb, kT_sb, v_sb, ot_sb,
                           wo_sb, y, ones1, onescol, tri, dbg, stages)
    nc.compile()
    return nc


def _emit_body(nc, tc, xT, wq_sb, wk_sb, wv_sb, bqt, bkt, bv_full, cosT_sb,
               sinSW_sb, qT_sb, kT_sb, v_sb, ot_sb, wo_sb, y, ones1,
               onescol, tri, dbg, stages="all"):
    with (
        tc.tile_pool(name="xs", bufs=4) as xs,
        tc.tile_pool(name="y_st", bufs=3) as ysp,
    ):
        for b in range(B):
            if stages in ("all", "s1"):
                _stage1(nc, tc, b, xT, wq_sb, wk_sb, wv_sb, bqt, bkt,
                        bv_full, cosT_sb, sinSW_sb, qT_sb, kT_sb, v_sb, xs)
            if dbg:
                for h in range(HPC):
                    nc.sync.dma_start(dbg["qT"].ap()[h, b], qT_sb[h][:])
                    nc.sync.dma_start(dbg["kT"].ap()[h, b], kT_sb[h][:])
                nc.sync.dma_start(
                    dbg["v"].ap()[b].rearrange("(c p) f -> p c f", p=128),
                    v_sb[:])
            if stages in ("all", "s23"):
                _stage23(nc, tc, b, qT_sb, kT_sb, v_sb, ones1, onescol, tri,
                         ot_sb, wo_sb, y, ysp, dbg)
        if dbg:
            for bb in range(B):
                for h in range(HPC):
                    nc.sync.dma_start(dbg["ot"].ap()[bb, h], ot_sb[bb][h][:])


def _stage1(nc, tc, b, xT, wq_sb, wk_sb, wv_sb, bqt, bkt, bv_full,
            cosT_sb, sinSW_sb, qT_sb, kT_sb, v_sb, xs):
    """QKV projection + RoPE for batch b: fills qT_sb/kT_sb/v_sb (bf16).

    Per 512-token tile: x^T streams in as two [128, 8, 512] halves. q/k are
    computed transposed ([feat, tok]) with W stationary, 8 same-bank matmuls
    per (projection, half). V is computed in natural [tok, feat] layout with
    the x chunk stationary and W_v moving (out = x_chunk.T @ W_v), 16
    same-bank matmuls per 128-token chunk - no transposes needed.
    """
    ks1 = os.environ.get("KS1", "full")
    with (
        tc.tile_pool(name="st", bufs=3) as st,
        tc.tile_pool(name="ps_qk", bufs=4, space="PSUM") as psqk,
        tc.tile_pool(name="ps_v", bufs=2, space="PSUM") as psv,
    ):
        for tau in range(NTT):
            pos = tau * TOKT
            gtok = b * T + pos
            accs = [psqk.tile([128, TOKT], F32, name="qk_acc") for _ in range(4)]
            xhalves = []
            for half in range(2):
                xt = xs.tile([128, 8, TOKT], BF16, name="xt")
                xhalves.append(xt)
                nc.sync.dma_start(
                    xt[:],
                    xT.ap()[half * 1024:(half + 1) * 1024, gtok:gtok + TOKT]
                    .rearrange("(c p) t -> p c t", p=128))
                for fi, (wsb, hh) in enumerate(
                        ((wq_sb, 0), (wq_sb, 1), (wk_sb, 0), (wk_sb, 1))):
                    for cl in range(8):
                        c = half * 8 + cl
                        nc.tensor.matmul(
                            accs[fi][:], wsb[:, c, hh * DH:(hh + 1) * DH],
                            xt[:, cl, :],
                            start=(c == 0), stop=(c == NDCH - 1))
            # V in natural layout: x chunk stationary, W_v moving
            if ks1 != "mm":
                for ch in range(4):
                    vps = psv.tile([128, HPC * DH], F32, name="v_ps")
                    for c in range(NDCH):
                        nc.tensor.matmul(
                            vps[:],
                            xhalves[c // 8][:, c % 8, ch * 128:(ch + 1) * 128],
                            wv_sb[:, c, :],
                            start=(c == 0), stop=(c == NDCH - 1))
                    nc.vector.tensor_tensor(
                        v_sb[:, tau * 4 + ch, :], vps[:], bv_full[:], ALU.add)
            if ks1 == "mm":
                continue
            # q/k evictions with bias (split ACT/DVE), then RoPE on DVE
            for fi, (bias, dest, hh) in enumerate(
                    ((bqt, qT_sb, 0), (bqt, qT_sb, 1),
                     (bkt, kT_sb, 0), (bkt, kT_sb, 1))):
                stg = st.tile([128, TOKT], BF16, name="stg")
                if fi < 2:
                    nc.scalar.activation(stg[:], accs[fi][:], AF.Identity,
                                         bias=bias[:, hh:hh + 1], scale=1.0)
                else:
                    nc.vector.tensor_scalar_add(stg[:], accs[fi][:],
                                                bias[:, hh:hh + 1])
                rot = st.tile([128, TOKT], BF16, name="rot")
                nc.scalar.copy(rot[0:64, :], stg[64:128, :])
                nc.vector.tensor_copy(rot[64:128, :], stg[0:64, :])
                nc.vector.tensor_tensor(
                    stg[:], stg[:], cosT_sb[:, pos:pos + TOKT], ALU.mult)
                nc.vector.tensor_tensor(
                    rot[:], rot[:], sinSW_sb[:, pos:pos + TOKT], ALU.mult)
                nc.vector.tensor_tensor(
                    dest[hh][:, pos:pos + TOKT], stg[:], rot[:], ALU.add)


def _stage23(nc, tc, b, qT_sb, kT_sb, v_sb, ones1, onescol, tri, ot_sb,
             wo_sb, y, ysp, dbg):
    """Causal attention for batch b + per-j output projection emission.

    kk-steps run in pairs. Per pair: S matmuls (full tiles split 256+256 into
    one bank) -> exp (ACT, bf16 out, scale folded) -> diag triangle zero (DVE
    mask multiply). The P@V and ones-rowsum accumulations for pair p-1 issue
    between the S matmuls of pair p, so each accumulator bank gets runs of 2
    and exp latency hides. After each j: reciprocal + PE-broadcast of
    1/rowsum, O^T normalized on DVE, then the output projection for these 512
    tokens (PSUM from the S pool).
    """
    with (
        tc.tile_pool(name="spsB", bufs=4, space="PSUM") as spsB,
        tc.tile_pool(name="rps", bufs=2, space="PSUM") as rps,
        tc.tile_pool(name="ops", bufs=2, space="PSUM") as ops,
        tc.tile_pool(name="scr", bufs=4) as scr,
        tc.tile_pool(name="pt_p", bufs=10) as ptp,
    ):
        for j in range(NJ):
            nkk = 4 * j + 4
            npair = nkk // 2
            rp = [rps.tile([1, 512], F32, name="r_ps") for _ in range(HPC)]
            op = [ops.tile([128, 512], F32, name="o_ps") for _ in range(HPC)]
            prev = None  # list of (h, [(pt, kk, off, n), ...])

            def emit_pv(plist):
                for h, pts in plist:
                    for pt, kk, off, n in pts:
                        nc.tensor.matmul(op[h][:, off:512],
                                         v_sb[:, kk, h * DH:(h + 1) * DH],
                                         pt[:, 0:n], start=(kk == 0),
                                         stop=(kk == nkk - 1))
                    for pt, kk, off, n in pts:
                        nc.tensor.matmul(rp[h][:, off:512], onescol[:],
                                         pt[:, 0:n], start=(kk == 0),
                                         stop=(kk == nkk - 1))

            for p in range(npair):
                cur = []
                for h in range(HPC):
                    qT, kT = qT_sb[h], kT_sb[h]
                    pts = []
                    for ki in range(2):
                        kk = p * 2 + ki
                        diag = (kk // 4 == j)
                        off = (kk % 4) * 128 if diag else 0
                        n = 512 - off
                        qlo = j * 512 + off
                        sp = spsB.tile([128, 512], F32, name="st_ps",
                                       tag="st_ps")
                        if n > 128:
                            nh = n // 2
                            nc.tensor.matmul(sp[:, 0:nh],
                                             kT[:, kk * 128:(kk + 1) * 128],
                                             qT[:, qlo:qlo + nh],
                                             start=True, stop=True)
                            nc.tensor.matmul(sp[:, nh:n],
                                             kT[:, kk * 128:(kk + 1) * 128],
                                             qT[:, qlo + nh:(j + 1) * 512],
                                             start=True, stop=True)
                        else:
                            nc.tensor.matmul(sp[:, 0:n],
                                             kT[:, kk * 128:(kk + 1) * 128],
                                             qT[:, qlo:(j + 1) * 512],
                                             start=True, stop=True)
                        pt = ptp.tile([128, 512], BF16, name="pt")
                        nc.scalar.activation(pt[:, 0:n], sp[:, 0:n], AF.Exp,
                                             bias=0.0, scale=SCALE)
                        if diag:
                            nc.vector.tensor_tensor(pt[:, 0:128], pt[:, 0:128],
                                                    tri[:], ALU.mult)
                        pts.append((pt, kk, off, n))
                    cur.append((h, pts))
                    if h == 0 and prev is not None:
                        emit_pv([prev[0]])
                if prev is not None:
                    emit_pv([prev[1]])
                if p == npair - 1:
                    emit_pv(cur)
                else:
                    prev = cur
            # rowsum -> reciprocal -> broadcast across partitions -> evict
            for h in range(HPC):
                rrow_inv = scr.tile([1, 512], BF16, name="rrow_inv")
                with nc.allow_low_precision(reason="softmax denom to bf16"):
                    nc.vector.reciprocal(rrow_inv[:], rp[h][:])
                rb_ps = spsB.tile([128, 512], F32, name="st_ps", tag="st_ps")
                nc.tensor.matmul(rb_ps[:], ones1[:], rrow_inv[:],
                                 start=True, stop=True)
                rb = scr.tile([128, 512], F32, name="rb")
                nc.scalar.copy(rb[:], rb_ps[:])
                nc.vector.tensor_tensor(ot_sb[b][h][:, j * 512:(j + 1) * 512],
                                        op[h][:], rb[:], ALU.mult)
            # ---- output projection for tokens [j*512, (j+1)*512) ----------
            for tt in range(4):
                trow = j * 4 + tt
                yst = ysp.tile([128, D_MODEL], BF16, name="y_st")
                for ft in range(NFT):
                    ps = spsB.tile([128, 512], F32, name="st_ps", tag="st_ps")
                    for h in range(HPC):
                        nc.tensor.matmul(
                            ps[:], ot_sb[b][h][:, trow * 128:(trow + 1) * 128],
                            wo_sb[:, h, ft * 512:(ft + 1) * 512],
                            start=(h == 0), stop=(h == HPC - 1))
                    if ft % 2 == 0:
                        nc.scalar.copy(yst[:, ft * 512:(ft + 1) * 512], ps[:])
                    else:
                        nc.vector.tensor_copy(yst[:, ft * 512:(ft + 1) * 512],
                                              ps[:])
                nc.sync.dma_start(
                    y.ap()[b * T + trow * 128:b * T + (trow + 1) * 128, :],
                    yst[:])


_CACHE = {}


def _get_nc():
    if "nc" not in _CACHE:
        _CACHE["nc"] = build_nc(debug=bool(int(os.environ.get("KERNEL_DEBUG", "0"))))
    return _CACHE["nc"]


def _host_prep(x, W_qkv, b_qkv, W_out, mask):
    from ml_dtypes import bfloat16
    xT = np.ascontiguousarray(x.reshape(BT, D_IN).T.astype(bfloat16))
    Wr = W_qkv.reshape(D_IN, H, 3, DH)
    br = b_qkv.reshape(H, 3, DH)
    # RoPE tables, transposed, sign-folded (rows 0:64 of sin negated) for the
    # half-swap rotate: q_rot = q*cos + swap_halves(q)*sinSW.
    inv_freq = (1.0 / (10000.0 ** (np.arange(0, DH, 2, dtype=np.float32) / DH))).astype(np.float32)
    tpos = np.arange(T, dtype=np.float32)
    freqs = tpos[:, None] * inv_freq[None, :]              # (T, 64)
    emb = np.concatenate([freqs, freqs], axis=-1)          # (T, 128)
    cosT = np.ascontiguousarray(np.cos(emb).astype(np.float32).T)
    sinSW = np.sin(emb).astype(np.float32).T               # (128, T)
    sinSW[0:64] = -sinSW[0:64]

    tri_m = (np.arange(128)[None, :] >= np.arange(128)[:, None]).astype(np.float32)
    ones1 = np.ones((1, 128), dtype=np.float32)
    onescol = np.ones((128, 1), dtype=np.float32)

    in_maps = []
    for i in range(NCORES):
        hs = [HPC * i + k for k in range(HPC)]
        bv_row = br[hs, 2, :].reshape(1, HPC * DH)
        in_maps.append({
            "xT": xT,
            "wq": np.ascontiguousarray(Wr[:, hs, 0, :].reshape(D_IN, HPC * DH).astype(bfloat16)),
            "wk": np.ascontiguousarray(Wr[:, hs, 1, :].reshape(D_IN, HPC * DH).astype(bfloat16)),
            "wv": np.ascontiguousarray(Wr[:, hs, 2, :].reshape(D_IN, HPC * DH).astype(bfloat16)),
            "bq": np.ascontiguousarray(br[hs, 0, :].reshape(HPC * DH)),
            "bk": np.ascontiguousarray(br[hs, 1, :].reshape(HPC * DH)),
            "bvf": np.ascontiguousarray(
                np.broadcast_to(bv_row, (128, HPC * DH)).astype(bfloat16)),
            "wo": np.ascontiguousarray(W_out[hs[0] * DH:(hs[-1] + 1) * DH, :].astype(bfloat16)),
            "cosT": np.ascontiguousarray(cosT.astype(bfloat16)),
            "sinSW": np.ascontiguousarray(sinSW.astype(bfloat16)),
            "tri": tri_m.astype(bfloat16),
            "ones1": ones1.astype(bfloat16),
            "onescol": onescol.astype(bfloat16),
        })
    return in_maps


def kernel(x, W_qkv, b_qkv, W_out, b_out, mask):
    x = np.asarray(x, dtype=np.float32)
    in_maps = _host_prep(np.asarray(x), np.asarray(W_qkv), np.asarray(b_qkv),
                         np.asarray(W_out), np.asarray(mask))
    nc = _get_nc()
    res = run_bass_kernel_spmd(nc, in_maps, core_ids=list(range(NCORES)))
    out = np.asarray(res.results[0]["y"], dtype=np.float32)
    for i in range(1, NCORES):
        out += np.asarray(res.results[i]["y"], dtype=np.float32)
    out += np.asarray(b_out, dtype=np.float32)[None, :]
    return out.reshape(B, T, D_MODEL).astype(np.float32)


# revision 22
# speedup vs baseline: 1.3754x; 1.0404x over previous
"""Multi-head causal attention (B=2, T=2048, D=2048, H=16) on 8 trn2 NeuronCores.

Sharding: tensor-parallel over heads (2 heads/core). x^T is replicated, W_qkv
column-sliced and W_out row-sliced per core; each core computes a full-shape
partial of the output projection and the host sums the 8 partials (+ b_out).

v3: bf16 end-to-end (fp32 PSUM accumulation; rel err ~7e-3 vs the 2e-2 gate)
and, critically, every PE matmul sequence is arranged to stay on one PSUM
bank for >=2 consecutive matmuls: HW measurement shows accumulation streams
that switch banks every matmul run at ~790ns/MM vs ~215-240ns for runs>=2
(unmodeled by CoreSim).

Stage 1 (QKV+RoPE): per 512-token tile, the four transposed projections
(q0,q1,k0,k1) run as 8-matmul same-bank bursts per half-x-tile; V is computed
directly in natural [token, feature] layout by making the x chunk the
stationary operand (out = x_chunk.T @ W_v slice), which kills the PE
transposes and their eviction copies entirely. Stage 2: scores transposed
[k, q], raw exp (scores < ~25, no max subtraction), denominators via
ones-column matmuls; kk-steps processed in pairs with P@V / rowsum matmuls
software-pipelined one pair behind the S-matmul+exp of the next pair, so PV
and rowsum hit their accumulator banks in runs of 2 while exp latency hides.
Full S tiles are split into two N=256 matmuls into the same bank. Diagonal
blocks shrink N to 512-128*(kk%4); the residual 128-col triangle is zeroed by
a DVE multiply with a constant mask. The output projection for each 512-token
block is emitted right after its O^T normalization, reusing the S PSUM pool.
Engine placement: ACT = exp + q bias + rotate-half copy + rb + half the y
evictions; DVE = RoPE muls, k bias, V/ot evictions, diag mask, reciprocal,
the other half of y; all DMA on SP; GpSimd unused (slow on HW, cannot read
PSUM).
"""

import math
import os

import numpy as np

import concourse.bass as bass
import concourse.mybir as mybir
import concourse.tile as tile
from concourse import bacc
from concourse.bass_utils import run_bass_kernel_spmd

B, T, D_IN, D_MODEL, H = 2, 2048, 2048, 2048, 16
DH = 128
NCORES = 8
HPC = H // NCORES  # heads per core
BT = B * T
SCALE = 1.0 / math.sqrt(DH)

F32 = mybir.dt.float32
BF16 = mybir.dt.bfloat16
AF = mybir.ActivationFunctionType
ALU = mybir.AluOpType

TOKT = 512             # stage-1 token tile
NTT = T // TOKT        # token tiles per batch (4)
NDCH = D_IN // 128     # d_in contraction chunks (16)
NQ = T // 128          # 128-token chunks per batch (16)
NJ = T // 512          # q 512-tiles per batch (4)
NFT = D_MODEL // 512   # output feature tiles (4)


def build_nc(debug=False, reps=1, stages=None):
    stages = stages or os.environ.get("KSTAGES", "all")
    nc = bacc.Bacc("TRN2", target_bir_lowering=False, debug=False,
                   num_devices=NCORES)

    xT = nc.dram_tensor("xT", [B, NTT, 2, 128, NDCH // 2, TOKT], BF16,
                        kind="ExternalInput")
    wq = nc.dram_tensor("wq", [D_IN, HPC * DH], BF16, kind="ExternalInput")
    wk = nc.dram_tensor("wk", [D_IN, HPC * DH], BF16, kind="ExternalInput")
    wv = nc.dram_tensor("wv", [D_IN, HPC * DH], BF16, kind="ExternalInput")
    bq = nc.dram_tensor("bq", [HPC * DH], F32, kind="ExternalInput")
    bk = nc.dram_tensor("bk", [HPC * DH], F32, kind="ExternalInput")
    bvf = nc.dram_tensor("bvf", [128, HPC * DH], BF16, kind="ExternalInput")
    wo = nc.dram_tensor("wo", [HPC * DH, D_MODEL], BF16, kind="ExternalInput")
    cosT = nc.dram_tensor("cosT", [DH, T], BF16, kind="ExternalInput")
    sinSW = nc.dram_tensor("sinSW", [DH, T], BF16, kind="ExternalInput")
    negid_d = nc.dram_tensor("negid", [128, 128], BF16, kind="ExternalInput")
    triB_d = nc.dram_tensor("triB", [128, 128], BF16, kind="ExternalInput")
    onesq_d = nc.dram_tensor("onesq", [128, 128], BF16, kind="ExternalInput")
    y = nc.dram_tensor("y", [BT, D_MODEL], BF16, kind="ExternalOutput")

    dbg = {}
    if debug:
        dbg["qT"] = nc.dram_tensor("dbg_qT", [HPC, B, DH, T], F32, kind="ExternalOutput")
        dbg["kT"] = nc.dram_tensor("dbg_kT", [HPC, B, DH, T], F32, kind="ExternalOutput")
        dbg["v"] = nc.dram_tensor("dbg_v", [B, T, HPC * DH], F32, kind="ExternalOutput")
        dbg["ot"] = nc.dram_tensor("dbg_ot", [B, HPC, DH, T], F32, kind="ExternalOutput")

    with tile.TileContext(nc) as tc:
        with (
            tc.tile_pool(name="persist", bufs=1) as pp,
            tc.tile_pool(name="weights", bufs=1) as wp,
            tc.tile_pool(name="qkv", bufs=1) as qp,
        ):
            # ---- per-core weights (persistent, outside the rep loop)
            wq_sb = wp.tile([128, NDCH, HPC * DH], BF16, name="wq_sb")
            wk_sb = wp.tile([128, NDCH, HPC * DH], BF16, name="wk_sb")
            wv_sb = wp.tile([128, NDCH, HPC * DH], BF16, name="wv_sb")
            wo_sb = wp.tile([128, HPC, D_MODEL], BF16, name="wo_sb")

            for hf in range(4):
                for t_, d_ in ((wq_sb, wq), (wk_sb, wk), (wv_sb, wv)):
                    nc.sync.dma_start(
                        t_[:, hf * (NDCH // 4):(hf + 1) * (NDCH // 4), :],
                        d_.ap()[hf * (D_IN // 4):(hf + 1) * (D_IN // 4), :]
                        .rearrange("(c p) f -> p c f", p=128))
            nc.sync.dma_start(wo_sb[:],
                              wo.ap().rearrange("(h p) f -> p h f", p=128))

            cosT_sb = pp.tile([DH, T], BF16, name="cosT_sb")
            sinSW_sb = pp.tile([DH, T], BF16, name="sinSW_sb")
            nc.sync.dma_start(cosT_sb[:], cosT.ap())
            nc.sync.dma_start(sinSW_sb[:], sinSW.ap())
            onesq = pp.tile([128, 128], BF16, name="onesq")
            negid = pp.tile([128, 128], BF16, name="negid")
            triB = pp.tile([128, 128], BF16, name="triB")
            nc.sync.dma_start(onesq[:], onesq_d.ap())
            nc.sync.dma_start(negid[:], negid_d.ap())
            nc.sync.dma_start(triB[:], triB_d.ap())
            bqt = pp.tile([128, HPC], F32, name="bqt")
            bkt = pp.tile([128, HPC], F32, name="bkt")
            bv_full = pp.tile([128, HPC * DH], BF16, name="bv_full")
            nc.sync.dma_start(bqt[:], bq.ap().rearrange("(h d) -> d h", d=DH))
            nc.sync.dma_start(bkt[:], bk.ap().rearrange("(h d) -> d h", d=DH))
            nc.sync.dma_start(bv_full[:], bvf.ap())

            # ---- per-batch Q^T/K^T/V and O^T buffers (persistent)
            qT_sb = [qp.tile([DH, T], BF16, name=f"qT{h}") for h in range(HPC)]
            kT_sb = [qp.tile([DH, T], BF16, name=f"kT{h}") for h in range(HPC)]
            v_sb = qp.tile([128, NQ, HPC * DH], BF16, name="v_sb")
            ot_sb = [[qp.tile([DH, T], BF16, name=f"ot{b}_{h}") for h in range(HPC)]
                     for b in range(B)]
            if stages == "s23":
                for h in range(HPC):
                    nc.gpsimd.memset(qT_sb[h][:], 0.5)
                    nc.gpsimd.memset(kT_sb[h][:], 0.5)
                nc.gpsimd.memset(v_sb[:], 0.5)

            import contextlib
            rep_ctx = (tc.For_i(0, reps, 1, hint_engines=(
                mybir.EngineType.PE, mybir.EngineType.Activation,
                mybir.EngineType.DVE, mybir.EngineType.Pool,
                mybir.EngineType.SP))
                if reps > 1 else contextlib.nullcontext())
            with rep_ctx:
                _emit_body(nc, tc, xT, wq_sb, wk_sb, wv_sb, bqt, bkt, bv_full,
                           cosT_sb, sinSW_sb, qT_sb, kT_sb, v_sb, ot_sb,
                           wo_sb, y, onesq, negid, triB, dbg, stages)
    nc.compile()
    return nc


def _emit_body(nc, tc, xT, wq_sb, wk_sb, wv_sb, bqt, bkt, bv_full, cosT_sb,
               sinSW_sb, qT_sb, kT_sb, v_sb, ot_sb, wo_sb, y, onesq,
               negid, triB, dbg, stages="all"):
    with (
        tc.tile_pool(name="xs", bufs=4) as xs,
        tc.tile_pool(name="y_st", bufs=3) as ysp,
    ):
        for b in range(B):
            if stages in ("all", "s1"):
                _stage1(nc, tc, b, xT, wq_sb, wk_sb, wv_sb, bqt, bkt,
                        bv_full, cosT_sb, sinSW_sb, qT_sb, kT_sb, v_sb, xs)
            if dbg:
                for h in range(HPC):
                    nc.sync.dma_start(dbg["qT"].ap()[h, b], qT_sb[h][:])
                    nc.sync.dma_start(dbg["kT"].ap()[h, b], kT_sb[h][:])
                nc.sync.dma_start(
                    dbg["v"].ap()[b].rearrange("(c p) f -> p c f", p=128),
                    v_sb[:])
            if stages in ("all", "s23"):
                _stage23(nc, tc, b, qT_sb, kT_sb, v_sb, onesq,
                         negid, triB, ot_sb, wo_sb, y, ysp, dbg)
        if dbg:
            for bb in range(B):
                for h in range(HPC):
                    nc.sync.dma_start(dbg["ot"].ap()[bb, h], ot_sb[bb][h][:])


def _stage1(nc, tc, b, xT, wq_sb, wk_sb, wv_sb, bqt, bkt, bv_full,
            cosT_sb, sinSW_sb, qT_sb, kT_sb, v_sb, xs):
    """QKV projection + RoPE for batch b: fills qT_sb/kT_sb/v_sb (bf16).

    x is host-pre-tiled so each [128, 8, 512] half-tile is one DMA with 8KB
    contiguous per partition (vs 512x1KB strided runs from plain x^T).

    Per 512-token tile: x^T streams in as two [128, 8, 512] halves. q/k are
    computed transposed ([feat, tok]) with W stationary, 8 same-bank matmuls
    per (projection, half). V is computed in natural [tok, feat] layout with
    the x chunk stationary and W_v moving (out = x_chunk.T @ W_v), 16
    same-bank matmuls per 128-token chunk - no transposes needed.
    """
    ks1 = os.environ.get("KS1", "full")
    with (
        tc.tile_pool(name="st", bufs=3) as st,
        tc.tile_pool(name="ps_qk", bufs=4, space="PSUM") as psqk,
        tc.tile_pool(name="ps_v", bufs=2, space="PSUM") as psv,
    ):
        for tau in range(NTT):
            pos = tau * TOKT
            gtok = b * T + pos
            accs = [psqk.tile([128, TOKT], F32, name="qk_acc") for _ in range(4)]
            xhalves = []
            for half in range(2):
                xt = xs.tile([128, 8, TOKT], BF16, name="xt")
                xhalves.append(xt)
                nc.sync.dma_start(xt[:], xT.ap()[b, tau, half])
                for fi, (wsb, hh) in enumerate(
                        ((wq_sb, 0), (wq_sb, 1), (wk_sb, 0), (wk_sb, 1))):
                    for cl in range(8):
                        c = half * 8 + cl
                        nc.tensor.matmul(
                            accs[fi][:], wsb[:, c, hh * DH:(hh + 1) * DH],
                            xt[:, cl, :],
                            start=(c == 0), stop=(c == NDCH - 1))
            # V in natural layout: x chunk stationary, W_v moving
            if ks1 != "mm":
                for ch in range(4):
                    vps = psv.tile([128, HPC * DH], F32, name="v_ps")
                    for c in range(NDCH):
                        nc.tensor.matmul(
                            vps[:],
                            xhalves[c // 8][:, c % 8, ch * 128:(ch + 1) * 128],
                            wv_sb[:, c, :],
                            start=(c == 0), stop=(c == NDCH - 1))
                    nc.vector.tensor_tensor(
                        v_sb[:, tau * 4 + ch, :], vps[:], bv_full[:], ALU.add)
            if ks1 == "mm":
                continue
            # q/k evictions with bias (split ACT/DVE), then RoPE on DVE
            for fi, (bias, dest, hh) in enumerate(
                    ((bqt, qT_sb, 0), (bqt, qT_sb, 1),
                     (bkt, kT_sb, 0), (bkt, kT_sb, 1))):
                stg = st.tile([128, TOKT], BF16, name="stg")
                if fi < 2:
                    nc.scalar.activation(stg[:], accs[fi][:], AF.Identity,
                                         bias=bias[:, hh:hh + 1], scale=1.0)
                else:
                    nc.vector.tensor_scalar_add(stg[:], accs[fi][:],
                                                bias[:, hh:hh + 1])
                rot = st.tile([128, TOKT], BF16, name="rot")
                nc.scalar.copy(rot[0:64, :], stg[64:128, :])
                nc.vector.tensor_copy(rot[64:128, :], stg[0:64, :])
                nc.vector.tensor_tensor(
                    stg[:], stg[:], cosT_sb[:, pos:pos + TOKT], ALU.mult)
                nc.vector.tensor_tensor(
                    rot[:], rot[:], sinSW_sb[:, pos:pos + TOKT], ALU.mult)
                nc.vector.tensor_tensor(
                    dest[hh][:, pos:pos + TOKT], stg[:], rot[:], ALU.add)


def _stage23(nc, tc, b, qT_sb, kT_sb, v_sb, onesq, negid, triB,
             ot_sb, wo_sb, y, ysp, dbg):
    ks23 = set(os.environ.get("KS23", "").split(","))
    """Causal attention for batch b + per-j output projection emission.

    kk-steps run in pairs. Per pair: S matmuls (full tiles split 256+256 into
    one bank) -> exp (ACT, bf16 out, scale folded) -> diag triangle zero (DVE
    mask multiply). The P@V and ones-rowsum accumulations for pair p-1 issue
    between the S matmuls of pair p, so each accumulator bank gets runs of 2
    and exp latency hides. After each j: reciprocal + PE-broadcast of
    1/rowsum, O^T normalized on DVE, then the output projection for these 512
    tokens (PSUM from the S pool).
    """
    with (
        tc.tile_pool(name="spsB", bufs=4, space="PSUM") as spsB,
        tc.tile_pool(name="rps", bufs=2, space="PSUM") as rps,
        tc.tile_pool(name="ops", bufs=2, space="PSUM") as ops,
        tc.tile_pool(name="scr", bufs=4) as scr,
        tc.tile_pool(name="pt_p", bufs=10) as ptp,
    ):
        def emit_y(jy):
            for tt in range(4):
                trow = jy * 4 + tt
                yst = ysp.tile([128, D_MODEL], BF16, name="y_st")
                for ft in range(NFT):
                    ps = spsB.tile([128, 512], F32, name="st_ps", tag="st_ps")
                    for h in range(HPC):
                        nc.tensor.matmul(
                            ps[:], ot_sb[b][h][:, trow * 128:(trow + 1) * 128],
                            wo_sb[:, h, ft * 512:(ft + 1) * 512],
                            start=(h == 0), stop=(h == HPC - 1))
                    nc.vector.tensor_copy(yst[:, ft * 512:(ft + 1) * 512],
                                          ps[:])
                # y goes out via the (otherwise idle) GpSimd SWDGE queue so
                # the SP queue stays free for the next batch's x reads
                nc.gpsimd.dma_start(
                    y.ap()[b * T + trow * 128:b * T + (trow + 1) * 128, :],
                    yst[:])

        pending_y = None
        for j in range(NJ):
            nkk = 4 * j + 4
            npair = nkk // 2
            rp = [rps.tile([128, 512], F32, name="r_ps") for _ in range(HPC)]
            op = [ops.tile([128, 512], F32, name="o_ps") for _ in range(HPC)]
            prev = None  # list of (h, [(pt, kk, off, n), ...])

            def emit_pv(plist):
                for h, pts in plist:
                    if "nopv" not in ks23:
                        for pt, kk, off, n in pts:
                            nc.tensor.matmul(op[h][:, off:512],
                                             v_sb[:, kk, h * DH:(h + 1) * DH],
                                             pt[:, 0:n], start=(kk == 0),
                                             stop=(kk == nkk - 1))
                    if "nors" not in ks23:
                        for pt, kk, off, n in pts:
                            nc.tensor.matmul(rp[h][:, off:512], onesq[:],
                                             pt[:, 0:n], start=(kk == 0),
                                             stop=(kk == nkk - 1))

            for p in range(npair):
                cur = []
                for h in range(HPC):
                    qT, kT = qT_sb[h], kT_sb[h]
                    pts = []
                    for ki in range(2):
                        kk = p * 2 + ki
                        diag = (kk // 4 == j)
                        off = (kk % 4) * 128 if diag else 0
                        n = 512 - off
                        qlo = j * 512 + off
                        sp = spsB.tile([128, 512], F32, name="st_ps",
                                       tag="st_ps")
                        if n > 128:
                            nh = n // 2
                            nc.tensor.matmul(sp[:, 0:nh],
                                             kT[:, kk * 128:(kk + 1) * 128],
                                             qT[:, qlo:qlo + nh],
                                             start=True, stop=not diag)
                            if diag:
                                # add -400 above the local diagonal (pre-exp
                                # mask); must run while the first half's
                                # has_written bits are still set
                                nc.tensor.matmul(sp[:, 0:128], negid[:],
                                                 triB[:], start=False,
                                                 stop=True)
                            nc.tensor.matmul(sp[:, nh:n],
                                             kT[:, kk * 128:(kk + 1) * 128],
                                             qT[:, qlo + nh:(j + 1) * 512],
                                             start=True, stop=True)
                        else:
                            nc.tensor.matmul(sp[:, 0:n],
                                             kT[:, kk * 128:(kk + 1) * 128],
                                             qT[:, qlo:(j + 1) * 512],
                                             start=True, stop=not diag)
                            if diag:
                                nc.tensor.matmul(sp[:, 0:128], negid[:],
                                                 triB[:], start=False,
                                                 stop=True)
                        pt = ptp.tile([128, 512], BF16, name="pt")
                        _af = AF.Identity if os.environ.get("KNOEXP") else AF.Exp
                        nc.scalar.activation(pt[:, 0:n], sp[:, 0:n], _af,
                                             bias=0.0, scale=SCALE)
                        pts.append((pt, kk, off, n))
                    cur.append((h, pts))
                    if h == 0 and prev is not None:
                        emit_pv([prev[0]])
                if prev is not None:
                    emit_pv([prev[1]])
                    prev = None
                if p == 0 and pending_y is not None and os.environ.get("KDEFY", "1") == "1":
                    if "noy" not in ks23:
                        emit_y(pending_y)
                    pending_y = None
                if p == npair - 1:
                    emit_pv(cur)
                else:
                    prev = cur
            # rowsum -> reciprocal -> broadcast across partitions -> evict
            for h in range(HPC):
                if "nors" in ks23 or "nopv" in ks23:
                    break
                rb = scr.tile([128, 512], F32, name="rb")
                with nc.allow_low_precision(reason="softmax denom"):
                    nc.vector.reciprocal(rb[:], rp[h][:])
                nc.vector.tensor_tensor(ot_sb[b][h][:, j * 512:(j + 1) * 512],
                                        op[h][:], rb[:], ALU.mult)
            if os.environ.get("KDEFY", "1") == "1":
                pending_y = j
            elif "noy" not in ks23:
                emit_y(j)
        if pending_y is not None and "noy" not in ks23:
            emit_y(NJ - 1)


_CACHE = {}


def _get_nc():
    if "nc" not in _CACHE:
        _CACHE["nc"] = build_nc(debug=bool(int(os.environ.get("KERNEL_DEBUG", "0"))))
    return _CACHE["nc"]


def _host_prep(x, W_qkv, b_qkv, W_out, mask):
    from ml_dtypes import bfloat16
    xTf = x.reshape(BT, D_IN).T          # [D_IN, BT]
    # tile to [b, tau, half, p, c, t] with per-partition-contiguous [c, t]
    xT = np.ascontiguousarray(
        xTf.reshape(2, 8, 128, B, NTT, TOKT)
        .transpose(3, 4, 0, 2, 1, 5).astype(bfloat16))
    Wr = W_qkv.reshape(D_IN, H, 3, DH)
    br = b_qkv.reshape(H, 3, DH)
    # RoPE tables, transposed, sign-folded (rows 0:64 of sin negated) for the
    # half-swap rotate: q_rot = q*cos + swap_halves(q)*sinSW.
    inv_freq = (1.0 / (10000.0 ** (np.arange(0, DH, 2, dtype=np.float32) / DH))).astype(np.float32)
    tpos = np.arange(T, dtype=np.float32)
    freqs = tpos[:, None] * inv_freq[None, :]              # (T, 64)
    emb = np.concatenate([freqs, freqs], axis=-1)          # (T, 128)
    cosT = np.ascontiguousarray(np.cos(emb).astype(np.float32).T)
    sinSW = np.sin(emb).astype(np.float32).T               # (128, T)
    sinSW[0:64] = -sinSW[0:64]

    negid = (-400.0 * np.eye(128)).astype(np.float32)
    triB = (np.arange(128)[None, :] < np.arange(128)[:, None]).astype(np.float32)
    onesq = np.ones((128, 128), dtype=np.float32)

    in_maps = []
    for i in range(NCORES):
        hs = [HPC * i + k for k in range(HPC)]
        bv_row = br[hs, 2, :].reshape(1, HPC * DH)
        in_maps.append({
            "xT": xT,
            "wq": np.ascontiguousarray(Wr[:, hs, 0, :].reshape(D_IN, HPC * DH).astype(bfloat16)),
            "wk": np.ascontiguousarray(Wr[:, hs, 1, :].reshape(D_IN, HPC * DH).astype(bfloat16)),
            "wv": np.ascontiguousarray(Wr[:, hs, 2, :].reshape(D_IN, HPC * DH).astype(bfloat16)),
            "bq": np.ascontiguousarray(br[hs, 0, :].reshape(HPC * DH)),
            "bk": np.ascontiguousarray(br[hs, 1, :].reshape(HPC * DH)),
            "bvf": np.ascontiguousarray(
                np.broadcast_to(bv_row, (128, HPC * DH)).astype(bfloat16)),
            "wo": np.ascontiguousarray(W_out[hs[0] * DH:(hs[-1] + 1) * DH, :].astype(bfloat16)),
            "cosT": np.ascontiguousarray(cosT.astype(bfloat16)),
            "sinSW": np.ascontiguousarray(sinSW.astype(bfloat16)),
            "negid": negid.astype(bfloat16),
            "triB": triB.astype(bfloat16),
            "onesq": onesq.astype(bfloat16),
        })
    return in_maps


def kernel(x, W_qkv, b_qkv, W_out, b_out, mask):
    x = np.asarray(x, dtype=np.float32)
    in_maps = _host_prep(np.asarray(x), np.asarray(W_qkv), np.asarray(b_qkv),
                         np.asarray(W_out), np.asarray(mask))
    nc = _get_nc()
    res = run_bass_kernel_spmd(nc, in_maps, core_ids=list(range(NCORES)))
    out = np.asarray(res.results[0]["y"], dtype=np.float32)
    for i in range(1, NCORES):
        out += np.asarray(res.results[i]["y"], dtype=np.float32)
    out += np.asarray(b_out, dtype=np.float32)[None, :]
    return out.reshape(B, T, D_MODEL).astype(np.float32)


# revision 25
# speedup vs baseline: 1.4256x; 1.0365x over previous
"""Multi-head causal attention (B=2, T=2048, D=2048, H=16) on 8 trn2 NeuronCores.

Sharding: tensor-parallel over heads (2 heads/core). x^T is replicated, W_qkv
column-sliced and W_out row-sliced per core; each core computes a full-shape
partial of the output projection and the host sums the 8 partials (+ b_out).

v3: bf16 end-to-end (fp32 PSUM accumulation; rel err ~7e-3 vs the 2e-2 gate)
and, critically, every PE matmul sequence is arranged to stay on one PSUM
bank for >=2 consecutive matmuls: HW measurement shows accumulation streams
that switch banks every matmul run at ~790ns/MM vs ~215-240ns for runs>=2
(unmodeled by CoreSim).

Stage 1 (QKV+RoPE): per 512-token tile, the four transposed projections
(q0,q1,k0,k1) run as 8-matmul same-bank bursts per half-x-tile; V is computed
directly in natural [token, feature] layout by making the x chunk the
stationary operand (out = x_chunk.T @ W_v slice), which kills the PE
transposes and their eviction copies entirely. Stage 2: scores transposed
[k, q], raw exp (scores < ~25, no max subtraction), denominators via
ones-column matmuls; kk-steps processed in pairs with P@V / rowsum matmuls
software-pipelined one pair behind the S-matmul+exp of the next pair, so PV
and rowsum hit their accumulator banks in runs of 2 while exp latency hides.
Full S tiles are split into two N=256 matmuls into the same bank. Diagonal
blocks shrink N to 512-128*(kk%4); the residual 128-col triangle is zeroed by
a DVE multiply with a constant mask. The output projection for each 512-token
block is emitted right after its O^T normalization, reusing the S PSUM pool.
Engine placement: ACT = exp + q bias + rotate-half copy + rb + half the y
evictions; DVE = RoPE muls, k bias, V/ot evictions, diag mask, reciprocal,
the other half of y; all DMA on SP; GpSimd unused (slow on HW, cannot read
PSUM).
"""

import math
import os

import numpy as np

import concourse.bass as bass
import concourse.mybir as mybir
import concourse.tile as tile
from concourse import bacc
from concourse.bass_utils import run_bass_kernel_spmd

B, T, D_IN, D_MODEL, H = 2, 2048, 2048, 2048, 16
DH = 128
NCORES = 8
HPC = H // NCORES  # heads per core
BT = B * T
SCALE = 1.0 / math.sqrt(DH)

F32 = mybir.dt.float32
BF16 = mybir.dt.bfloat16
AF = mybir.ActivationFunctionType
ALU = mybir.AluOpType

TOKT = 512             # stage-1 token tile
NTT = T // TOKT        # token tiles per batch (4)
NDCH = D_IN // 128     # d_in contraction chunks (16)
NQ = T // 128          # 128-token chunks per batch (16)
NJ = T // 512          # q 512-tiles per batch (4)
NFT = D_MODEL // 512   # output feature tiles (4)


def build_nc(debug=False, reps=1, stages=None):
    stages = stages or os.environ.get("KSTAGES", "all")
    nc = bacc.Bacc("TRN2", target_bir_lowering=False, debug=False,
                   num_devices=NCORES)

    xT = nc.dram_tensor("xT", [B, NTT, 2, 128, NDCH // 2, TOKT], BF16,
                        kind="ExternalInput")
    wq = nc.dram_tensor("wq", [D_IN, HPC * DH], BF16, kind="ExternalInput")
    wk = nc.dram_tensor("wk", [D_IN, HPC * DH], BF16, kind="ExternalInput")
    wv = nc.dram_tensor("wv", [D_IN, HPC * DH], BF16, kind="ExternalInput")
    bq = nc.dram_tensor("bq", [HPC * DH], F32, kind="ExternalInput")
    bk = nc.dram_tensor("bk", [HPC * DH], F32, kind="ExternalInput")
    bvf = nc.dram_tensor("bvf", [128, HPC * DH], BF16, kind="ExternalInput")
    wo = nc.dram_tensor("wo", [HPC * DH, D_MODEL], BF16, kind="ExternalInput")
    cosT = nc.dram_tensor("cosT", [DH, T], BF16, kind="ExternalInput")
    sinSW = nc.dram_tensor("sinSW", [DH, T], BF16, kind="ExternalInput")
    negid_d = nc.dram_tensor("negid", [128, 128], BF16, kind="ExternalInput")
    triB_d = nc.dram_tensor("triB", [128, 128], BF16, kind="ExternalInput")
    onesq_d = nc.dram_tensor("onesq", [128, 128], BF16, kind="ExternalInput")
    y = nc.dram_tensor("y", [BT, D_MODEL], BF16, kind="ExternalOutput")

    dbg = {}
    if debug:
        dbg["qT"] = nc.dram_tensor("dbg_qT", [HPC, B, DH, T], F32, kind="ExternalOutput")
        dbg["kT"] = nc.dram_tensor("dbg_kT", [HPC, B, DH, T], F32, kind="ExternalOutput")
        dbg["v"] = nc.dram_tensor("dbg_v", [B, T, HPC * DH], F32, kind="ExternalOutput")
        dbg["ot"] = nc.dram_tensor("dbg_ot", [B, HPC, DH, T], F32, kind="ExternalOutput")

    with tile.TileContext(nc) as tc:
        with (
            tc.tile_pool(name="persist", bufs=1) as pp,
            tc.tile_pool(name="weights", bufs=1) as wp,
            tc.tile_pool(name="qkv", bufs=1) as qp,
        ):
            # ---- per-core weights (persistent, outside the rep loop)
            wq_sb = wp.tile([128, NDCH, HPC * DH], BF16, name="wq_sb")
            wk_sb = wp.tile([128, NDCH, HPC * DH], BF16, name="wk_sb")
            wv_sb = wp.tile([128, NDCH, HPC * DH], BF16, name="wv_sb")
            wo_sb = wp.tile([128, HPC, D_MODEL], BF16, name="wo_sb")

            for hf in range(4):
                for t_, d_ in ((wq_sb, wq), (wk_sb, wk), (wv_sb, wv)):
                    nc.sync.dma_start(
                        t_[:, hf * (NDCH // 4):(hf + 1) * (NDCH // 4), :],
                        d_.ap()[hf * (D_IN // 4):(hf + 1) * (D_IN // 4), :]
                        .rearrange("(c p) f -> p c f", p=128))
            nc.sync.dma_start(wo_sb[:],
                              wo.ap().rearrange("(h p) f -> p h f", p=128))

            cosT_sb = pp.tile([DH, T], BF16, name="cosT_sb")
            sinSW_sb = pp.tile([DH, T], BF16, name="sinSW_sb")
            nc.sync.dma_start(cosT_sb[:], cosT.ap())
            nc.sync.dma_start(sinSW_sb[:], sinSW.ap())
            onesq = pp.tile([128, 128], BF16, name="onesq")
            negid = pp.tile([128, 128], BF16, name="negid")
            triB = pp.tile([128, 128], BF16, name="triB")
            nc.sync.dma_start(onesq[:], onesq_d.ap())
            nc.sync.dma_start(negid[:], negid_d.ap())
            nc.sync.dma_start(triB[:], triB_d.ap())
            bqt = pp.tile([128, HPC], F32, name="bqt")
            bkt = pp.tile([128, HPC], F32, name="bkt")
            bv_full = pp.tile([128, HPC * DH], BF16, name="bv_full")
            nc.sync.dma_start(bqt[:], bq.ap().rearrange("(h d) -> d h", d=DH))
            nc.sync.dma_start(bkt[:], bk.ap().rearrange("(h d) -> d h", d=DH))
            nc.sync.dma_start(bv_full[:], bvf.ap())

            # ---- per-batch Q^T/K^T/V and O^T buffers (persistent)
            qT_sb = [qp.tile([DH, T], BF16, name=f"qT{h}") for h in range(HPC)]
            kT_sb = [qp.tile([DH, T], BF16, name=f"kT{h}") for h in range(HPC)]
            v_sb = qp.tile([128, NQ, HPC * DH], BF16, name="v_sb")
            ot_sb = [[qp.tile([DH, T], BF16, name=f"ot{b}_{h}") for h in range(HPC)]
                     for b in range(B)]
            if stages == "s23":
                for h in range(HPC):
                    nc.gpsimd.memset(qT_sb[h][:], 0.5)
                    nc.gpsimd.memset(kT_sb[h][:], 0.5)
                nc.gpsimd.memset(v_sb[:], 0.5)

            import contextlib
            rep_ctx = (tc.For_i(0, reps, 1, hint_engines=(
                mybir.EngineType.PE, mybir.EngineType.Activation,
                mybir.EngineType.DVE, mybir.EngineType.Pool,
                mybir.EngineType.SP))
                if reps > 1 else contextlib.nullcontext())
            with rep_ctx:
                _emit_body(nc, tc, xT, wq_sb, wk_sb, wv_sb, bqt, bkt, bv_full,
                           cosT_sb, sinSW_sb, qT_sb, kT_sb, v_sb, ot_sb,
                           wo_sb, y, onesq, negid, triB, dbg, stages)
    nc.compile()
    return nc


def _emit_body(nc, tc, xT, wq_sb, wk_sb, wv_sb, bqt, bkt, bv_full, cosT_sb,
               sinSW_sb, qT_sb, kT_sb, v_sb, ot_sb, wo_sb, y, onesq,
               negid, triB, dbg, stages="all"):
    with (
        tc.tile_pool(name="xs", bufs=4) as xs,
        tc.tile_pool(name="y_st", bufs=3) as ysp,
    ):
        carry_y = None
        for b in range(B):
            if stages in ("all", "s1"):
                carry_y = _stage1(nc, tc, b, xT, wq_sb, wk_sb, wv_sb, bqt,
                                  bkt, bv_full, cosT_sb, sinSW_sb, qT_sb,
                                  kT_sb, v_sb, xs, inject=carry_y)
            if dbg:
                for h in range(HPC):
                    nc.sync.dma_start(dbg["qT"].ap()[h, b], qT_sb[h][:])
                    nc.sync.dma_start(dbg["kT"].ap()[h, b], kT_sb[h][:])
                nc.sync.dma_start(
                    dbg["v"].ap()[b].rearrange("(c p) f -> p c f", p=128),
                    v_sb[:])
            if stages in ("all", "s23"):
                prefetch = None
                if b + 1 < B and stages == "all":
                    pf_tiles = [xs.tile([128, 8, TOKT], BF16, name="xt")
                                for _ in range(2)]

                    def prefetch(bn=b + 1, tiles=pf_tiles):
                        for half in range(2):
                            nc.sync.dma_start(tiles[half][:],
                                              xT.ap()[bn, 0, half])
                    prefetch.tiles = pf_tiles
                carry_y = _stage23(nc, tc, b, qT_sb, kT_sb, v_sb, onesq,
                                   negid, triB, ot_sb, wo_sb, y, ysp, dbg,
                                   prefetch=prefetch,
                                   defer=(prefetch is not None))
                if carry_y is not None:
                    carry_y = (carry_y, prefetch.tiles)
        if dbg:
            for bb in range(B):
                for h in range(HPC):
                    nc.sync.dma_start(dbg["ot"].ap()[bb, h], ot_sb[bb][h][:])


def _stage1(nc, tc, b, xT, wq_sb, wk_sb, wv_sb, bqt, bkt, bv_full,
            cosT_sb, sinSW_sb, qT_sb, kT_sb, v_sb, xs, inject=None):
    """QKV projection + RoPE for batch b: fills qT_sb/kT_sb/v_sb (bf16).

    x is host-pre-tiled so each [128, 8, 512] half-tile is one DMA with 8KB
    contiguous per partition (vs 512x1KB strided runs from plain x^T).

    Per 512-token tile: x^T streams in as two [128, 8, 512] halves. q/k are
    computed transposed ([feat, tok]) with W stationary, 8 same-bank matmuls
    per (projection, half). V is computed in natural [tok, feat] layout with
    the x chunk stationary and W_v moving (out = x_chunk.T @ W_v), 16
    same-bank matmuls per 128-token chunk - no transposes needed.
    """
    ks1 = os.environ.get("KS1", "full")
    inj_y, pre_tiles = (inject if inject is not None else (None, None))
    with (
        tc.tile_pool(name="st", bufs=3) as st,
        tc.tile_pool(name="ps_qk", bufs=4, space="PSUM") as psqk,
        tc.tile_pool(name="ps_v", bufs=2, space="PSUM") as psv,
    ):
        for tau in range(NTT):
            pos = tau * TOKT
            gtok = b * T + pos
            accs = [psqk.tile([128, TOKT], F32, name="qk_acc") for _ in range(4)]
            xhalves = []
            for half in range(2):
                if tau == 0 and pre_tiles is not None:
                    xt = pre_tiles[half]
                else:
                    xt = xs.tile([128, 8, TOKT], BF16, name="xt")
                    nc.sync.dma_start(xt[:], xT.ap()[b, tau, half])
                xhalves.append(xt)
                for fi, (wsb, hh) in enumerate(
                        ((wq_sb, 0), (wq_sb, 1), (wk_sb, 0), (wk_sb, 1))):
                    for cl in range(8):
                        c = half * 8 + cl
                        nc.tensor.matmul(
                            accs[fi][:], wsb[:, c, hh * DH:(hh + 1) * DH],
                            xt[:, cl, :],
                            start=(c == 0), stop=(c == NDCH - 1))
            if tau == 0 and inj_y is not None:
                inj_y(psv)
            # V in natural layout: x chunk stationary, W_v moving
            if ks1 != "mm":
                for ch in range(4):
                    vps = psv.tile([128, HPC * DH], F32, name="v_ps")
                    for c in range(NDCH):
                        nc.tensor.matmul(
                            vps[:],
                            xhalves[c // 8][:, c % 8, ch * 128:(ch + 1) * 128],
                            wv_sb[:, c, :],
                            start=(c == 0), stop=(c == NDCH - 1))
                    nc.vector.tensor_tensor(
                        v_sb[:, tau * 4 + ch, :], vps[:], bv_full[:], ALU.add)
            if ks1 == "mm":
                continue
            # q/k evictions with bias (split ACT/DVE), then RoPE on DVE
            for fi, (bias, dest, hh) in enumerate(
                    ((bqt, qT_sb, 0), (bqt, qT_sb, 1),
                     (bkt, kT_sb, 0), (bkt, kT_sb, 1))):
                stg = st.tile([128, TOKT], BF16, name="stg")
                if fi < 2:
                    nc.scalar.activation(stg[:], accs[fi][:], AF.Identity,
                                         bias=bias[:, hh:hh + 1], scale=1.0)
                else:
                    nc.vector.tensor_scalar_add(stg[:], accs[fi][:],
                                                bias[:, hh:hh + 1])
                rot = st.tile([128, TOKT], BF16, name="rot")
                nc.scalar.copy(rot[0:64, :], stg[64:128, :])
                nc.vector.tensor_copy(rot[64:128, :], stg[0:64, :])
                nc.vector.tensor_tensor(
                    stg[:], stg[:], cosT_sb[:, pos:pos + TOKT], ALU.mult)
                nc.vector.tensor_tensor(
                    rot[:], rot[:], sinSW_sb[:, pos:pos + TOKT], ALU.mult)
                nc.vector.tensor_tensor(
                    dest[hh][:, pos:pos + TOKT], stg[:], rot[:], ALU.add)


def _stage23(nc, tc, b, qT_sb, kT_sb, v_sb, onesq, negid, triB,
             ot_sb, wo_sb, y, ysp, dbg, prefetch=None, defer=False):
    ks23 = set(os.environ.get("KS23", "").split(","))
    """Causal attention for batch b + per-j output projection emission.

    kk-steps run in pairs. Per pair: S matmuls (full tiles split 256+256 into
    one bank) -> exp (ACT, bf16 out, scale folded) -> diag triangle zero (DVE
    mask multiply). The P@V and ones-rowsum accumulations for pair p-1 issue
    between the S matmuls of pair p, so each accumulator bank gets runs of 2
    and exp latency hides. After each j: reciprocal + PE-broadcast of
    1/rowsum, O^T normalized on DVE, then the output projection for these 512
    tokens (PSUM from the S pool).
    """
    with (
        tc.tile_pool(name="spsB", bufs=4, space="PSUM") as spsB,
        tc.tile_pool(name="rps", bufs=2, space="PSUM") as rps,
        tc.tile_pool(name="ops", bufs=2, space="PSUM") as ops,
        tc.tile_pool(name="scr", bufs=4) as scr,
        tc.tile_pool(name="pt_p", bufs=10) as ptp,
    ):
        def emit_y(jy, pool=None):
            for tt in range(4):
                trow = jy * 4 + tt
                yst = ysp.tile([128, D_MODEL], BF16, name="y_st")
                for ft in range(NFT):
                    if pool is None:
                        ps = spsB.tile([128, 512], F32, name="st_ps",
                                       tag="st_ps")
                    else:
                        ps = pool.tile([128, 512], F32, name="y_ps")
                    for h in range(HPC):
                        nc.tensor.matmul(
                            ps[:], ot_sb[b][h][:, trow * 128:(trow + 1) * 128],
                            wo_sb[:, h, ft * 512:(ft + 1) * 512],
                            start=(h == 0), stop=(h == HPC - 1))
                    nc.vector.tensor_copy(yst[:, ft * 512:(ft + 1) * 512],
                                          ps[:])
                # y goes out via the (otherwise idle) GpSimd SWDGE queue so
                # the SP queue stays free for the next batch's x reads
                nc.gpsimd.dma_start(
                    y.ap()[b * T + trow * 128:b * T + (trow + 1) * 128, :],
                    yst[:])

        pending_y = None
        for j in range(NJ):
            nkk = 4 * j + 4
            npair = nkk // 2
            rp = [rps.tile([128, 512], F32, name="r_ps") for _ in range(HPC)]
            op = [ops.tile([128, 512], F32, name="o_ps") for _ in range(HPC)]
            prev = None  # list of (h, [(pt, kk, off, n), ...])

            def emit_pv(plist):
                for h, pts in plist:
                    if "nopv" not in ks23:
                        for pt, kk, off, n in pts:
                            nc.tensor.matmul(op[h][:, off:512],
                                             v_sb[:, kk, h * DH:(h + 1) * DH],
                                             pt[:, 0:n], start=(kk == 0),
                                             stop=(kk == nkk - 1))
                    if "nors" not in ks23:
                        for pt, kk, off, n in pts:
                            nc.tensor.matmul(rp[h][:, off:512], onesq[:],
                                             pt[:, 0:n], start=(kk == 0),
                                             stop=(kk == nkk - 1))

            for p in range(npair):
                cur = []
                for h in range(HPC):
                    qT, kT = qT_sb[h], kT_sb[h]
                    pts = []
                    for ki in range(2):
                        kk = p * 2 + ki
                        diag = (kk // 4 == j)
                        off = (kk % 4) * 128 if diag else 0
                        n = 512 - off
                        qlo = j * 512 + off
                        sp = spsB.tile([128, 512], F32, name="st_ps",
                                       tag="st_ps")
                        if n > 128:
                            nh = n // 2
                            nc.tensor.matmul(sp[:, 0:nh],
                                             kT[:, kk * 128:(kk + 1) * 128],
                                             qT[:, qlo:qlo + nh],
                                             start=True, stop=not diag)
                            if diag:
                                # add -400 above the local diagonal (pre-exp
                                # mask); must run while the first half's
                                # has_written bits are still set
                                nc.tensor.matmul(sp[:, 0:128], negid[:],
                                                 triB[:], start=False,
                                                 stop=True)
                            nc.tensor.matmul(sp[:, nh:n],
                                             kT[:, kk * 128:(kk + 1) * 128],
                                             qT[:, qlo + nh:(j + 1) * 512],
                                             start=True, stop=True)
                        else:
                            nc.tensor.matmul(sp[:, 0:n],
                                             kT[:, kk * 128:(kk + 1) * 128],
                                             qT[:, qlo:(j + 1) * 512],
                                             start=True, stop=not diag)
                            if diag:
                                nc.tensor.matmul(sp[:, 0:128], negid[:],
                                                 triB[:], start=False,
                                                 stop=True)
                        pt = ptp.tile([128, 512], BF16, name="pt")
                        _af = AF.Identity if os.environ.get("KNOEXP") else AF.Exp
                        nc.scalar.activation(pt[:, 0:n], sp[:, 0:n], _af,
                                             bias=0.0, scale=SCALE)
                        pts.append((pt, kk, off, n))
                    cur.append((h, pts))
                    if h == 0 and prev is not None:
                        emit_pv([prev[0]])
                if prev is not None:
                    emit_pv([prev[1]])
                    prev = None
                if p == 0 and pending_y is not None and os.environ.get("KDEFY", "1") == "1":
                    if "noy" not in ks23:
                        emit_y(pending_y)
                    pending_y = None
                if p == npair - 1:
                    emit_pv(cur)
                else:
                    prev = cur
            # rowsum -> reciprocal -> broadcast across partitions -> evict
            for h in range(HPC):
                if "nors" in ks23 or "nopv" in ks23:
                    break
                rb = scr.tile([128, 512], F32, name="rb")
                with nc.allow_low_precision(reason="softmax denom"):
                    nc.vector.reciprocal(rb[:], rp[h][:])
                nc.vector.tensor_tensor(ot_sb[b][h][:, j * 512:(j + 1) * 512],
                                        op[h][:], rb[:], ALU.mult)
            if os.environ.get("KDEFY", "1") == "1":
                pending_y = j
            elif "noy" not in ks23:
                emit_y(j)
            if j == NJ - 2 and prefetch is not None:
                prefetch()
        if pending_y is not None and "noy" not in ks23:
            if defer:
                return lambda pool: emit_y(NJ - 1, pool)
            emit_y(NJ - 1)
        return None


_CACHE = {}


def _get_nc():
    if "nc" not in _CACHE:
        _CACHE["nc"] = build_nc(debug=bool(int(os.environ.get("KERNEL_DEBUG", "0"))))
    return _CACHE["nc"]


def _host_prep(x, W_qkv, b_qkv, W_out, mask):
    from ml_dtypes import bfloat16
    xTf = x.reshape(BT, D_IN).T          # [D_IN, BT]
    # tile to [b, tau, half, p, c, t] with per-partition-contiguous [c, t]
    xT = np.ascontiguousarray(
        xTf.reshape(2, 8, 128, B, NTT, TOKT)
        .transpose(3, 4, 0, 2, 1, 5).astype(bfloat16))
    Wr = W_qkv.reshape(D_IN, H, 3, DH)
    br = b_qkv.reshape(H, 3, DH)
    # RoPE tables, transposed, sign-folded (rows 0:64 of sin negated) for the
    # half-swap rotate: q_rot = q*cos + swap_halves(q)*sinSW.
    inv_freq = (1.0 / (10000.0 ** (np.arange(0, DH, 2, dtype=np.float32) / DH))).astype(np.float32)
    tpos = np.arange(T, dtype=np.float32)
    freqs = tpos[:, None] * inv_freq[None, :]              # (T, 64)
    emb = np.concatenate([freqs, freqs], axis=-1)          # (T, 128)
    cosT = np.ascontiguousarray(np.cos(emb).astype(np.float32).T)
    sinSW = np.sin(emb).astype(np.float32).T               # (128, T)
    sinSW[0:64] = -sinSW[0:64]

    negid = (-400.0 * np.eye(128)).astype(np.float32)
    triB = (np.arange(128)[None, :] < np.arange(128)[:, None]).astype(np.float32)
    onesq = np.ones((128, 128), dtype=np.float32)

    in_maps = []
    for i in range(NCORES):
        hs = [HPC * i + k for k in range(HPC)]
        bv_row = br[hs, 2, :].reshape(1, HPC * DH)
        in_maps.append({
            "xT": xT,
            "wq": np.ascontiguousarray(Wr[:, hs, 0, :].reshape(D_IN, HPC * DH).astype(bfloat16)),
            "wk": np.ascontiguousarray(Wr[:, hs, 1, :].reshape(D_IN, HPC * DH).astype(bfloat16)),
            "wv": np.ascontiguousarray(Wr[:, hs, 2, :].reshape(D_IN, HPC * DH).astype(bfloat16)),
            "bq": np.ascontiguousarray(br[hs, 0, :].reshape(HPC * DH)),
            "bk": np.ascontiguousarray(br[hs, 1, :].reshape(HPC * DH)),
            "bvf": np.ascontiguousarray(
                np.broadcast_to(bv_row, (128, HPC * DH)).astype(bfloat16)),
            "wo": np.ascontiguousarray(W_out[hs[0] * DH:(hs[-1] + 1) * DH, :].astype(bfloat16)),
            "cosT": np.ascontiguousarray(cosT.astype(bfloat16)),
            "sinSW": np.ascontiguousarray(sinSW.astype(bfloat16)),
            "negid": negid.astype(bfloat16),
            "triB": triB.astype(bfloat16),
            "onesq": onesq.astype(bfloat16),
        })
    return in_maps


def kernel(x, W_qkv, b_qkv, W_out, b_out, mask):
    x = np.asarray(x, dtype=np.float32)
    in_maps = _host_prep(np.asarray(x), np.asarray(W_qkv), np.asarray(b_qkv),
                         np.asarray(W_out), np.asarray(mask))
    nc = _get_nc()
    res = run_bass_kernel_spmd(nc, in_maps, core_ids=list(range(NCORES)))
    out = np.asarray(res.results[0]["y"], dtype=np.float32)
    for i in range(1, NCORES):
        out += np.asarray(res.results[i]["y"], dtype=np.float32)
    out += np.asarray(b_out, dtype=np.float32)[None, :]
    return out.reshape(B, T, D_MODEL).astype(np.float32)


# revision 26
# speedup vs baseline: 1.4732x; 1.0334x over previous
"""Multi-head causal attention (B=2, T=2048, D=2048, H=16) on 8 trn2 NeuronCores.

Sharding: tensor-parallel over heads (2 heads/core). x^T is replicated, W_qkv
column-sliced and W_out row-sliced per core; each core computes a full-shape
partial of the output projection and the host sums the 8 partials (+ b_out).

v3: bf16 end-to-end (fp32 PSUM accumulation; rel err ~7e-3 vs the 2e-2 gate)
and, critically, every PE matmul sequence is arranged to stay on one PSUM
bank for >=2 consecutive matmuls: HW measurement shows accumulation streams
that switch banks every matmul run at ~790ns/MM vs ~215-240ns for runs>=2
(unmodeled by CoreSim).

Stage 1 (QKV+RoPE): per 512-token tile, the four transposed projections
(q0,q1,k0,k1) run as 8-matmul same-bank bursts per half-x-tile; V is computed
directly in natural [token, feature] layout by making the x chunk the
stationary operand (out = x_chunk.T @ W_v slice), which kills the PE
transposes and their eviction copies entirely. Stage 2: scores transposed
[k, q], raw exp (scores < ~25, no max subtraction), denominators via
ones-column matmuls; kk-steps processed in pairs with P@V / rowsum matmuls
software-pipelined one pair behind the S-matmul+exp of the next pair, so PV
and rowsum hit their accumulator banks in runs of 2 while exp latency hides.
Full S tiles are split into two N=256 matmuls into the same bank. Diagonal
blocks shrink N to 512-128*(kk%4); the residual 128-col triangle is zeroed by
a DVE multiply with a constant mask. The output projection for each 512-token
block is emitted right after its O^T normalization, reusing the S PSUM pool.
Engine placement: ACT = exp + q bias + rotate-half copy + rb + half the y
evictions; DVE = RoPE muls, k bias, V/ot evictions, diag mask, reciprocal,
the other half of y; all DMA on SP; GpSimd unused (slow on HW, cannot read
PSUM).
"""

import math
import os

import numpy as np

import concourse.bass as bass
import concourse.mybir as mybir
import concourse.tile as tile
from concourse import bacc
from concourse.bass_utils import run_bass_kernel_spmd

B, T, D_IN, D_MODEL, H = 2, 2048, 2048, 2048, 16
DH = 128
NCORES = 8
HPC = H // NCORES  # heads per core
BT = B * T
SCALE = 1.0 / math.sqrt(DH)

F32 = mybir.dt.float32
BF16 = mybir.dt.bfloat16
AF = mybir.ActivationFunctionType
ALU = mybir.AluOpType

TOKT = 512             # stage-1 token tile
NTT = T // TOKT        # token tiles per batch (4)
NDCH = D_IN // 128     # d_in contraction chunks (16)
NQ = T // 128          # 128-token chunks per batch (16)
NJ = T // 512          # q 512-tiles per batch (4)
NFT = D_MODEL // 512   # output feature tiles (4)


def build_nc(debug=False, reps=1, stages=None):
    stages = stages or os.environ.get("KSTAGES", "all")
    nc = bacc.Bacc("TRN2", target_bir_lowering=False, debug=False,
                   num_devices=NCORES)

    xT = nc.dram_tensor("xT", [B, NTT, 2, 128, NDCH // 2, TOKT], BF16,
                        kind="ExternalInput")
    wq = nc.dram_tensor("wq", [D_IN, HPC * DH], BF16, kind="ExternalInput")
    wk = nc.dram_tensor("wk", [D_IN, HPC * DH], BF16, kind="ExternalInput")
    wv = nc.dram_tensor("wv", [D_IN, HPC * DH], BF16, kind="ExternalInput")
    bq = nc.dram_tensor("bq", [HPC * DH], F32, kind="ExternalInput")
    bk = nc.dram_tensor("bk", [HPC * DH], F32, kind="ExternalInput")
    bvf = nc.dram_tensor("bvf", [128, HPC * DH], BF16, kind="ExternalInput")
    wo = nc.dram_tensor("wo", [HPC * DH, D_MODEL], BF16, kind="ExternalInput")
    cosT = nc.dram_tensor("cosT", [DH, T], BF16, kind="ExternalInput")
    sinSW = nc.dram_tensor("sinSW", [DH, T], BF16, kind="ExternalInput")
    negid_d = nc.dram_tensor("negid", [128, 128], BF16, kind="ExternalInput")
    triB_d = nc.dram_tensor("triB", [128, 128], BF16, kind="ExternalInput")
    onesq_d = nc.dram_tensor("onesq", [128, 128], BF16, kind="ExternalInput")
    y = nc.dram_tensor("y", [BT, D_MODEL], BF16, kind="ExternalOutput")

    dbg = {}
    if debug:
        dbg["qT"] = nc.dram_tensor("dbg_qT", [HPC, B, DH, T], F32, kind="ExternalOutput")
        dbg["kT"] = nc.dram_tensor("dbg_kT", [HPC, B, DH, T], F32, kind="ExternalOutput")
        dbg["v"] = nc.dram_tensor("dbg_v", [B, T, HPC * DH], F32, kind="ExternalOutput")
        dbg["ot"] = nc.dram_tensor("dbg_ot", [B, HPC, DH, T], F32, kind="ExternalOutput")

    with tile.TileContext(nc) as tc:
        with (
            tc.tile_pool(name="persist", bufs=1) as pp,
            tc.tile_pool(name="weights", bufs=1) as wp,
            tc.tile_pool(name="qkv", bufs=1) as qp,
        ):
            # ---- per-core weights (persistent, outside the rep loop)
            wq_sb = wp.tile([128, NDCH, HPC * DH], BF16, name="wq_sb")
            wk_sb = wp.tile([128, NDCH, HPC * DH], BF16, name="wk_sb")
            wv_sb = wp.tile([128, NDCH, HPC * DH], BF16, name="wv_sb")
            wo_sb = wp.tile([128, HPC, D_MODEL], BF16, name="wo_sb")

            for hf in range(4):
                for t_, d_ in ((wq_sb, wq), (wk_sb, wk), (wv_sb, wv)):
                    nc.sync.dma_start(
                        t_[:, hf * (NDCH // 4):(hf + 1) * (NDCH // 4), :],
                        d_.ap()[hf * (D_IN // 4):(hf + 1) * (D_IN // 4), :]
                        .rearrange("(c p) f -> p c f", p=128))
            nc.sync.dma_start(wo_sb[:],
                              wo.ap().rearrange("(h p) f -> p h f", p=128))

            cosT_sb = pp.tile([DH, T], BF16, name="cosT_sb")
            sinSW_sb = pp.tile([DH, T], BF16, name="sinSW_sb")
            nc.sync.dma_start(cosT_sb[:], cosT.ap())
            nc.sync.dma_start(sinSW_sb[:], sinSW.ap())
            onesq = pp.tile([128, 128], BF16, name="onesq")
            negid = pp.tile([128, 128], BF16, name="negid")
            triB = pp.tile([128, 128], BF16, name="triB")
            nc.sync.dma_start(onesq[:], onesq_d.ap())
            nc.sync.dma_start(negid[:], negid_d.ap())
            nc.sync.dma_start(triB[:], triB_d.ap())
            bqt = pp.tile([128, HPC], F32, name="bqt")
            bkt = pp.tile([128, HPC], F32, name="bkt")
            bv_full = pp.tile([128, HPC * DH], BF16, name="bv_full")
            nc.sync.dma_start(bqt[:], bq.ap().rearrange("(h d) -> d h", d=DH))
            nc.sync.dma_start(bkt[:], bk.ap().rearrange("(h d) -> d h", d=DH))
            nc.sync.dma_start(bv_full[:], bvf.ap())

            # ---- per-batch Q^T/K^T/V and O^T buffers (persistent)
            qT_sb = [qp.tile([DH, T], BF16, name=f"qT{h}") for h in range(HPC)]
            kT_sb = [qp.tile([DH, T], BF16, name=f"kT{h}") for h in range(HPC)]
            v_sb = qp.tile([128, NQ, HPC * DH], BF16, name="v_sb")
            ot_sb = [[qp.tile([DH, T], BF16, name=f"ot{b}_{h}") for h in range(HPC)]
                     for b in range(B)]
            if stages == "s23":
                for h in range(HPC):
                    nc.gpsimd.memset(qT_sb[h][:], 0.5)
                    nc.gpsimd.memset(kT_sb[h][:], 0.5)
                nc.gpsimd.memset(v_sb[:], 0.5)

            import contextlib
            rep_ctx = (tc.For_i(0, reps, 1, hint_engines=(
                mybir.EngineType.PE, mybir.EngineType.Activation,
                mybir.EngineType.DVE, mybir.EngineType.Pool,
                mybir.EngineType.SP))
                if reps > 1 else contextlib.nullcontext())
            with rep_ctx:
                _emit_body(nc, tc, xT, wq_sb, wk_sb, wv_sb, bqt, bkt, bv_full,
                           cosT_sb, sinSW_sb, qT_sb, kT_sb, v_sb, ot_sb,
                           wo_sb, y, onesq, negid, triB, dbg, stages)
    nc.compile()
    return nc


def _emit_body(nc, tc, xT, wq_sb, wk_sb, wv_sb, bqt, bkt, bv_full, cosT_sb,
               sinSW_sb, qT_sb, kT_sb, v_sb, ot_sb, wo_sb, y, onesq,
               negid, triB, dbg, stages="all"):
    with (
        tc.tile_pool(name="xs", bufs=4) as xs,
        tc.tile_pool(name="y_st", bufs=3) as ysp,
    ):
        carry_y = None
        for b in range(B):
            if stages in ("all", "s1"):
                carry_y = _stage1(nc, tc, b, xT, wq_sb, wk_sb, wv_sb, bqt,
                                  bkt, bv_full, cosT_sb, sinSW_sb, qT_sb,
                                  kT_sb, v_sb, xs, inject=carry_y)
            if dbg:
                for h in range(HPC):
                    nc.sync.dma_start(dbg["qT"].ap()[h, b], qT_sb[h][:])
                    nc.sync.dma_start(dbg["kT"].ap()[h, b], kT_sb[h][:])
                nc.sync.dma_start(
                    dbg["v"].ap()[b].rearrange("(c p) f -> p c f", p=128),
                    v_sb[:])
            if stages in ("all", "s23"):
                prefetch = None
                if b + 1 < B and stages == "all":
                    pf_tiles = [xs.tile([128, 8, TOKT], BF16, name="xt")
                                for _ in range(2)]

                    def prefetch(bn=b + 1, tiles=pf_tiles):
                        for half in range(2):
                            nc.sync.dma_start(tiles[half][:],
                                              xT.ap()[bn, 0, half])
                    prefetch.tiles = pf_tiles
                carry_y = _stage23(nc, tc, b, qT_sb, kT_sb, v_sb, onesq,
                                   negid, triB, ot_sb, wo_sb, y, ysp, dbg,
                                   prefetch=prefetch,
                                   defer=(prefetch is not None))
                if carry_y is not None:
                    carry_y = (carry_y, prefetch.tiles)
        if dbg:
            for bb in range(B):
                for h in range(HPC):
                    nc.sync.dma_start(dbg["ot"].ap()[bb, h], ot_sb[bb][h][:])


def _stage1(nc, tc, b, xT, wq_sb, wk_sb, wv_sb, bqt, bkt, bv_full,
            cosT_sb, sinSW_sb, qT_sb, kT_sb, v_sb, xs, inject=None):
    """QKV projection + RoPE for batch b: fills qT_sb/kT_sb/v_sb (bf16).

    x is host-pre-tiled so each [128, 8, 512] half-tile is one DMA with 8KB
    contiguous per partition (vs 512x1KB strided runs from plain x^T).

    Per 512-token tile: x^T streams in as two [128, 8, 512] halves. q/k are
    computed transposed ([feat, tok]) with W stationary, 8 same-bank matmuls
    per (projection, half). V is computed in natural [tok, feat] layout with
    the x chunk stationary and W_v moving (out = x_chunk.T @ W_v), 16
    same-bank matmuls per 128-token chunk - no transposes needed.
    """
    ks1 = os.environ.get("KS1", "full")
    inj_y, pre_tiles = (inject if inject is not None else (None, None))
    with (
        tc.tile_pool(name="st", bufs=3) as st,
        tc.tile_pool(name="ps_qk", bufs=4, space="PSUM") as psqk,
        tc.tile_pool(name="ps_v", bufs=2, space="PSUM") as psv,
    ):
        for tau in range(NTT):
            pos = tau * TOKT
            gtok = b * T + pos
            accs = [psqk.tile([128, TOKT], F32, name="qk_acc") for _ in range(4)]
            xhalves = []
            for half in range(2):
                if tau == 0 and pre_tiles is not None:
                    xt = pre_tiles[half]
                else:
                    xt = xs.tile([128, 8, TOKT], BF16, name="xt")
                    nc.sync.dma_start(xt[:], xT.ap()[b, tau, half])
                xhalves.append(xt)
                for fi, (wsb, hh) in enumerate(
                        ((wq_sb, 0), (wq_sb, 1), (wk_sb, 0), (wk_sb, 1))):
                    for cl in range(8):
                        c = half * 8 + cl
                        nc.tensor.matmul(
                            accs[fi][:], wsb[:, c, hh * DH:(hh + 1) * DH],
                            xt[:, cl, :],
                            start=(c == 0), stop=(c == NDCH - 1))
            if tau == 0 and inj_y is not None:
                inj_y(psv)
            # V in natural layout: x chunk stationary, W_v moving
            if ks1 != "mm":
                for ch in range(4):
                    vps = psv.tile([128, HPC * DH], F32, name="v_ps")
                    for c in range(NDCH):
                        nc.tensor.matmul(
                            vps[:],
                            xhalves[c // 8][:, c % 8, ch * 128:(ch + 1) * 128],
                            wv_sb[:, c, :],
                            start=(c == 0), stop=(c == NDCH - 1))
                    nc.vector.tensor_tensor(
                        v_sb[:, tau * 4 + ch, :], vps[:], bv_full[:], ALU.add)
            if ks1 == "mm":
                continue
            # q/k evictions with bias (split ACT/DVE), then RoPE on DVE
            for fi, (bias, dest, hh) in enumerate(
                    ((bqt, qT_sb, 0), (bqt, qT_sb, 1),
                     (bkt, kT_sb, 0), (bkt, kT_sb, 1))):
                stg = st.tile([128, TOKT], BF16, name="stg")
                if fi < 2:
                    nc.scalar.activation(stg[:], accs[fi][:], AF.Identity,
                                         bias=bias[:, hh:hh + 1], scale=1.0)
                else:
                    nc.vector.tensor_scalar_add(stg[:], accs[fi][:],
                                                bias[:, hh:hh + 1])
                rot = st.tile([128, TOKT], BF16, name="rot")
                nc.scalar.copy(rot[0:64, :], stg[64:128, :])
                nc.vector.tensor_copy(rot[64:128, :], stg[0:64, :])
                nc.vector.tensor_tensor(
                    stg[:], stg[:], cosT_sb[:, pos:pos + TOKT], ALU.mult)
                nc.vector.tensor_tensor(
                    rot[:], rot[:], sinSW_sb[:, pos:pos + TOKT], ALU.mult)
                nc.vector.tensor_tensor(
                    dest[hh][:, pos:pos + TOKT], stg[:], rot[:], ALU.add)


def _stage23(nc, tc, b, qT_sb, kT_sb, v_sb, onesq, negid, triB,
             ot_sb, wo_sb, y, ysp, dbg, prefetch=None, defer=False):
    ks23 = set(os.environ.get("KS23", "").split(","))
    """Causal attention for batch b + per-j output projection emission.

    kk-steps run in pairs. Per pair: S matmuls (full tiles split 256+256 into
    one bank) -> exp (ACT, bf16 out, scale folded) -> diag triangle zero (DVE
    mask multiply). The P@V and ones-rowsum accumulations for pair p-1 issue
    between the S matmuls of pair p, so each accumulator bank gets runs of 2
    and exp latency hides. After each j: reciprocal + PE-broadcast of
    1/rowsum, O^T normalized on DVE, then the output projection for these 512
    tokens (PSUM from the S pool).
    """
    with (
        tc.tile_pool(name="spsB", bufs=4, space="PSUM") as spsB,
        tc.tile_pool(name="rps", bufs=2, space="PSUM") as rps,
        tc.tile_pool(name="ops", bufs=2, space="PSUM") as ops,
        tc.tile_pool(name="scr", bufs=4) as scr,
        tc.tile_pool(name="pt_p", bufs=14) as ptp,
    ):
        def emit_y(jy, pool=None):
            for tt in range(4):
                trow = jy * 4 + tt
                yst = ysp.tile([128, D_MODEL], BF16, name="y_st")
                for ft in range(NFT):
                    if pool is None:
                        ps = spsB.tile([128, 512], F32, name="st_ps",
                                       tag="st_ps")
                    else:
                        ps = pool.tile([128, 512], F32, name="y_ps")
                    for h in range(HPC):
                        nc.tensor.matmul(
                            ps[:], ot_sb[b][h][:, trow * 128:(trow + 1) * 128],
                            wo_sb[:, h, ft * 512:(ft + 1) * 512],
                            start=(h == 0), stop=(h == HPC - 1))
                    nc.vector.tensor_copy(yst[:, ft * 512:(ft + 1) * 512],
                                          ps[:])
                # y goes out via the (otherwise idle) GpSimd SWDGE queue so
                # the SP queue stays free for the next batch's x reads
                nc.gpsimd.dma_start(
                    y.ap()[b * T + trow * 128:b * T + (trow + 1) * 128, :],
                    yst[:])

        pending_y = None
        for j in range(NJ):
            nkk = 4 * j + 4
            npair = nkk // 2
            rp = [rps.tile([128, 512], F32, name="r_ps") for _ in range(HPC)]
            op = [ops.tile([128, 512], F32, name="o_ps") for _ in range(HPC)]
            prev = None  # list of (h, [(pt, kk, off, n), ...])

            rs_pend = [[] for _ in range(HPC)]

            def flush_rs(h, force=False):
                if "nors" in ks23:
                    rs_pend[h].clear()
                    return
                if len(rs_pend[h]) >= 4 or (force and rs_pend[h]):
                    for pt, kk, off, n in rs_pend[h]:
                        nc.tensor.matmul(rp[h][:, off:512], onesq[:],
                                         pt[:, 0:n], start=(kk == 0),
                                         stop=(kk == nkk - 1))
                    rs_pend[h].clear()

            def emit_pv(plist):
                for h, pts in plist:
                    if "nopv" not in ks23:
                        for pt, kk, off, n in pts:
                            nc.tensor.matmul(op[h][:, off:512],
                                             v_sb[:, kk, h * DH:(h + 1) * DH],
                                             pt[:, 0:n], start=(kk == 0),
                                             stop=(kk == nkk - 1))
                    rs_pend[h].extend(pts)
                    flush_rs(h)

            for p in range(npair):
                cur = []
                for h in range(HPC):
                    qT, kT = qT_sb[h], kT_sb[h]
                    pts = []
                    for ki in range(2):
                        kk = p * 2 + ki
                        diag = (kk // 4 == j)
                        off = (kk % 4) * 128 if diag else 0
                        n = 512 - off
                        qlo = j * 512 + off
                        sp = spsB.tile([128, 512], F32, name="st_ps",
                                       tag="st_ps")
                        if n > 128:
                            nh = n // 2
                            nc.tensor.matmul(sp[:, 0:nh],
                                             kT[:, kk * 128:(kk + 1) * 128],
                                             qT[:, qlo:qlo + nh],
                                             start=True, stop=not diag)
                            if diag:
                                # add -400 above the local diagonal (pre-exp
                                # mask); must run while the first half's
                                # has_written bits are still set
                                nc.tensor.matmul(sp[:, 0:128], negid[:],
                                                 triB[:], start=False,
                                                 stop=True)
                            nc.tensor.matmul(sp[:, nh:n],
                                             kT[:, kk * 128:(kk + 1) * 128],
                                             qT[:, qlo + nh:(j + 1) * 512],
                                             start=True, stop=True)
                        else:
                            nc.tensor.matmul(sp[:, 0:n],
                                             kT[:, kk * 128:(kk + 1) * 128],
                                             qT[:, qlo:(j + 1) * 512],
                                             start=True, stop=not diag)
                            if diag:
                                nc.tensor.matmul(sp[:, 0:128], negid[:],
                                                 triB[:], start=False,
                                                 stop=True)
                        pt = ptp.tile([128, 512], BF16, name="pt")
                        _af = AF.Identity if os.environ.get("KNOEXP") else AF.Exp
                        nc.scalar.activation(pt[:, 0:n], sp[:, 0:n], _af,
                                             bias=0.0, scale=SCALE)
                        pts.append((pt, kk, off, n))
                    cur.append((h, pts))
                    if h == 0 and prev is not None:
                        emit_pv([prev[0]])
                if prev is not None:
                    emit_pv([prev[1]])
                    prev = None
                if p == 0 and pending_y is not None and os.environ.get("KDEFY", "1") == "1":
                    if "noy" not in ks23:
                        emit_y(pending_y)
                    pending_y = None
                if p == npair - 1:
                    emit_pv(cur)
                else:
                    prev = cur
            for h in range(HPC):
                flush_rs(h, force=True)
            # rowsum -> reciprocal -> evict
            for h in range(HPC):
                if "nors" in ks23 or "nopv" in ks23:
                    break
                rb = scr.tile([128, 512], F32, name="rb")
                with nc.allow_low_precision(reason="softmax denom"):
                    nc.vector.reciprocal(rb[:], rp[h][:])
                nc.vector.tensor_tensor(ot_sb[b][h][:, j * 512:(j + 1) * 512],
                                        op[h][:], rb[:], ALU.mult)
            if os.environ.get("KDEFY", "1") == "1":
                pending_y = j
            elif "noy" not in ks23:
                emit_y(j)
            if j == NJ - 2 and prefetch is not None:
                prefetch()
        if pending_y is not None and "noy" not in ks23:
            if defer:
                return lambda pool: emit_y(NJ - 1, pool)
            emit_y(NJ - 1)
        return None


_CACHE = {}


def _get_nc():
    if "nc" not in _CACHE:
        _CACHE["nc"] = build_nc(debug=bool(int(os.environ.get("KERNEL_DEBUG", "0"))))
    return _CACHE["nc"]


def _host_prep(x, W_qkv, b_qkv, W_out, mask):
    from ml_dtypes import bfloat16
    xTf = x.reshape(BT, D_IN).T          # [D_IN, BT]
    # tile to [b, tau, half, p, c, t] with per-partition-contiguous [c, t]
    xT = np.ascontiguousarray(
        xTf.reshape(2, 8, 128, B, NTT, TOKT)
        .transpose(3, 4, 0, 2, 1, 5).astype(bfloat16))
    Wr = W_qkv.reshape(D_IN, H, 3, DH)
    br = b_qkv.reshape(H, 3, DH)
    # RoPE tables, transposed, sign-folded (rows 0:64 of sin negated) for the
    # half-swap rotate: q_rot = q*cos + swap_halves(q)*sinSW.
    inv_freq = (1.0 / (10000.0 ** (np.arange(0, DH, 2, dtype=np.float32) / DH))).astype(np.float32)
    tpos = np.arange(T, dtype=np.float32)
    freqs = tpos[:, None] * inv_freq[None, :]              # (T, 64)
    emb = np.concatenate([freqs, freqs], axis=-1)          # (T, 128)
    cosT = np.ascontiguousarray(np.cos(emb).astype(np.float32).T)
    sinSW = np.sin(emb).astype(np.float32).T               # (128, T)
    sinSW[0:64] = -sinSW[0:64]

    negid = (-400.0 * np.eye(128)).astype(np.float32)
    triB = (np.arange(128)[None, :] < np.arange(128)[:, None]).astype(np.float32)
    onesq = np.ones((128, 128), dtype=np.float32)

    in_maps = []
    for i in range(NCORES):
        hs = [HPC * i + k for k in range(HPC)]
        bv_row = br[hs, 2, :].reshape(1, HPC * DH)
        in_maps.append({
            "xT": xT,
            "wq": np.ascontiguousarray(Wr[:, hs, 0, :].reshape(D_IN, HPC * DH).astype(bfloat16)),
            "wk": np.ascontiguousarray(Wr[:, hs, 1, :].reshape(D_IN, HPC * DH).astype(bfloat16)),
            "wv": np.ascontiguousarray(Wr[:, hs, 2, :].reshape(D_IN, HPC * DH).astype(bfloat16)),
            "bq": np.ascontiguousarray(br[hs, 0, :].reshape(HPC * DH)),
            "bk": np.ascontiguousarray(br[hs, 1, :].reshape(HPC * DH)),
            "bvf": np.ascontiguousarray(
                np.broadcast_to(bv_row, (128, HPC * DH)).astype(bfloat16)),
            "wo": np.ascontiguousarray(W_out[hs[0] * DH:(hs[-1] + 1) * DH, :].astype(bfloat16)),
            "cosT": np.ascontiguousarray(cosT.astype(bfloat16)),
            "sinSW": np.ascontiguousarray(sinSW.astype(bfloat16)),
            "negid": negid.astype(bfloat16),
            "triB": triB.astype(bfloat16),
            "onesq": onesq.astype(bfloat16),
        })
    return in_maps


def kernel(x, W_qkv, b_qkv, W_out, b_out, mask):
    x = np.asarray(x, dtype=np.float32)
    in_maps = _host_prep(np.asarray(x), np.asarray(W_qkv), np.asarray(b_qkv),
                         np.asarray(W_out), np.asarray(mask))
    nc = _get_nc()
    res = run_bass_kernel_spmd(nc, in_maps, core_ids=list(range(NCORES)))
    out = np.asarray(res.results[0]["y"], dtype=np.float32)
    for i in range(1, NCORES):
        out += np.asarray(res.results[i]["y"], dtype=np.float32)
    out += np.asarray(b_out, dtype=np.float32)[None, :]
    return out.reshape(B, T, D_MODEL).astype(np.float32)
